# revision 1
# baseline (speedup 1.0000x reference)
"""Trainium2 Bass kernel for nn_Block_9268539425531 (MLA transformer block).

Sharding: 2 batch groups x 4-way TP within each group of 4 cores.
Per core (b = core//4, r = core%4, heads H = [4r, 4r+4)):
  Phase A: ln1 + w_down on own token slice (512 tokens), AllGather h+dkv.
  Phase B: q/k/v/qR/kR projections for own 4 heads, all 2048 tokens,
           spilled to DRAM.
  Phase C: causal attention for own 4 heads (scoresT layout, matmul-based
           partition softmax reductions), AllGather oT.
  Phase D: w_o + residual + ln2 on own token slice.
  Phase E: FFN (full hidden dim, own token slice) + residual.
All matmuls in float32r (full-rate fp32, ~1.6e-4 rel rounding per matmul).

Measured on 8 axon-tunneled trn2 cores: relative error 2.8e-4 vs the fp32
jax reference; cost-model (TimelineSim) single-rep device time ~0.98 ms/core
(PE busy ~0.74 ms = compute roofline for fp32r), wall-clock K-diff measures
are tunnel-jitter-limited (1.8-3.1 ms/rep bounds).
"""
import math
import numpy as np

B, T, C = 2, 2048, 2048
NH = 16
DK = 128
DHR = 64
LAT = 512
P = 128
NT = 512           # tokens per core
CC = C // P        # 16
NCORES = 8
SCALE = 1.0 / math.sqrt(DK)
NEG = -1.0e9
RG = [[0, 1, 2, 3], [4, 5, 6, 7]]

_CACHE = {}


# ---------------------------------------------------------------- program ---
def build_program(repeat=1, nocc=False, stop_after=None):
    from contextlib import ExitStack
    from concourse import bass, bacc, tile, mybir

    dt = mybir.dt
    f32 = dt.float32
    f32r = dt.float32r
    AF = mybir.ActivationFunctionType
    OP = mybir.AluOpType

    nc = bacc.Bacc("TRN2", target_bir_lowering=False, debug=False,
                   num_devices=NCORES)

    def din(name, shape, dtype=f32r):
        return nc.dram_tensor(name, shape, dtype, kind="ExternalInput")

    xT_d = din("xT", [CC, P, NT])
    ln1s_d = din("ln1s", [P, CC], f32)
    ln1b_d = din("ln1b", [P, CC], f32)
    ln2s_d = din("ln2s", [P, CC], f32)
    ln2b_d = din("ln2b", [P, CC], f32)
    wdown_d = din("wdown", [CC, P, 8 * P])
    bdown_d = din("bdown", [P, 8], f32)
    wqr_d = din("wqr", [CC, P, 2 * P])
    bqr_d = din("bqr", [P, 2], f32)
    wkr_d = din("wkr", [CC, P, P])
    bkr_d = din("bkr", [P, 1], f32)
    r2_d = din("r2", [P, P])
    cosq_d = din("cosq", [2, 4, P, NT], f32)
    sinq_d = din("sinq", [2, 4, P, NT], f32)
    cosk_d = din("cosk", [4, P, NT], f32)
    sink_d = din("sink", [4, P, NT], f32)
    wuk_d = din("wuk", [4, P, 4 * P])
    buk_d = din("buk", [P, 4], f32)
    wuv_d = din("wuv", [4, P, 4 * P])
    buv_d = din("buv", [P, 4], f32)
    wuq_d = din("wuq", [4, P, 4 * P])
    buq_d = din("buq", [P, 4], f32)
    mask_d = din("mask", [4, P, NT], f32)
    ones_r_d = din("ones_r", [P, P])
    wo_d = din("wo", [CC, 4, P, 4 * P])
    bo_d = din("bo", [P, CC], f32)
    wff1_d = din("wff1", [CC, 16, P, 4 * P])
    bff1_d = din("bff1", [P, 64], f32)
    wff2_d = din("wff2", [4, CC, P, CC * P])
    bff2_d = din("bff2", [P, CC], f32)
    outT_d = nc.dram_tensor("outT", [CC, P, NT], f32, kind="ExternalOutput")

    with tile.TileContext(nc) as tc, ExitStack() as ctx:
        pc = ctx.enter_context(tc.tile_pool(name="const", bufs=1))
        pdram = ctx.enter_context(tc.tile_pool(name="dram", bufs=1, space="DRAM"))

        # ---- small constants resident for the whole kernel (~3KB/part)
        ones_r = pc.tile([P, P], f32r)
        nc.sync.dma_start(ones_r[:], ones_r_d[:])
        r2 = pc.tile([P, P], f32r)
        nc.sync.dma_start(r2[:], r2_d[:])
        ln1s = pc.tile([P, CC], f32)
        nc.sync.dma_start(ln1s[:], ln1s_d[:])
        ln1b = pc.tile([P, CC], f32)
        nc.sync.dma_start(ln1b[:], ln1b_d[:])
        ln2s = pc.tile([P, CC], f32)
        nc.sync.dma_start(ln2s[:], ln2s_d[:])
        ln2b = pc.tile([P, CC], f32)
        nc.sync.dma_start(ln2b[:], ln2b_d[:])
        bdown = pc.tile([P, 8], f32)
        nc.sync.dma_start(bdown[:], bdown_d[:])
        bqr = pc.tile([P, 2], f32)
        nc.sync.dma_start(bqr[:], bqr_d[:])
        bkr = pc.tile([P, 1], f32)
        nc.sync.dma_start(bkr[:], bkr_d[:])
        buk = pc.tile([P, 4], f32)
        nc.sync.dma_start(buk[:], buk_d[:])
        buv = pc.tile([P, 4], f32)
        nc.sync.dma_start(buv[:], buv_d[:])
        buq = pc.tile([P, 4], f32)
        nc.sync.dma_start(buq[:], buq_d[:])
        bo = pc.tile([P, CC], f32)
        nc.sync.dma_start(bo[:], bo_d[:])
        bff1 = pc.tile([P, 64], f32)
        nc.sync.dma_start(bff1[:], bff1_d[:])
        bff2 = pc.tile([P, CC], f32)
        nc.sync.dma_start(bff2[:], bff2_d[:])
        eps_t = pc.tile([P, 1], f32)
        nc.vector.memset(eps_t[:], 1e-6)

        agin1 = pdram.tile([24, P, NT], f32r)         # 16 h + 8 dkv chunks
        agout1h = pdram.tile([4, CC, P, NT], f32r)
        agout1d = pdram.tile([4, 8, P, NT], f32r)
        agin2 = pdram.tile([4, P, T], f32r)           # own-heads oT
        agout2 = pdram.tile([16, P, T], f32r)
        qR_sp = pdram.tile([2, P, T], f32r)           # projection spills
        kR_sp = pdram.tile([P, T], f32r)
        v_sp = pdram.tile([16, P, 4 * P], f32r)

        pid = nc.sync.partition_id()
        colo = (pid % 4) * NT

        def layer_norm(src_tiles, pstream, pstat, pool_ps, lns, lnb,
                       out_slices, out_name, rep):
            """src [16][P, NT] -> normalized f32r slices (list of APs)."""
            ps_mean = pool_ps.tile([P, NT], f32, name=f"lnpm{rep}{out_name}")
            ps_sq = pool_ps.tile([P, NT], f32, name=f"lnps{rep}{out_name}")
            for ci in range(CC):
                sq = pstream.tile([P, NT], f32r, name="lnsq", tag="lnsq")
                nc.scalar.square(sq[:], src_tiles[ci])
                nc.tensor.matmul(ps_mean[:], ones_r[:], src_tiles[ci],
                                 start=(ci == 0), stop=(ci == CC - 1),
                                 skip_group_check=True)
                nc.tensor.matmul(ps_sq[:], ones_r[:], sq[:],
                                 start=(ci == 0), stop=(ci == CC - 1),
                                 skip_group_check=True)
            meanb = pstat.tile([P, NT], f32, name="lnmean", tag="lnmean")
            nc.vector.tensor_scalar_mul(meanb[:], ps_mean[:], 1.0 / C)
            m2 = pstat.tile([P, NT], f32, name="lnm2", tag="lnm2")
            nc.vector.tensor_mul(m2[:], meanb[:], meanb[:])
            var = pstat.tile([P, NT], f32, name="lnvar", tag="lnvar")
            nc.vector.scalar_tensor_tensor(var[:], ps_sq[:], 1.0 / C, m2[:],
                                           OP.mult, OP.subtract)
            std = pstat.tile([P, NT], f32, name="lnstd", tag="lnstd")
            nc.scalar.activation(std[:], var[:], AF.Sqrt, bias=eps_t[:])
            rstd = pstat.tile([P, NT], f32, name="lnrstd", tag="lnrstd")
            nc.vector.reciprocal(rstd[:], std[:])
            outs = []
            for ci in range(CC):
                eng = nc.gpsimd if ci % 3 == 2 else nc.vector
                t1 = pstream.tile([P, NT], f32, name="lnt1", tag="lnt1")
                eng.tensor_sub(t1[:], src_tiles[ci], meanb[:])
                t2 = pstream.tile([P, NT], f32, name="lnt2", tag="lnt2")
                eng.tensor_mul(t2[:], t1[:], rstd[:])
                h = out_slices[ci]
                eng.tensor_scalar(h, t2[:], lns[:, ci:ci + 1],
                                  lnb[:, ci:ci + 1], OP.mult, OP.add)
                outs.append(h)
            return outs

        _ph = ["A", "B1", "B2", "C", "D", "E"]
        _upto = len(_ph) if stop_after is None else _ph.index(stop_after) + 1
        _en = set(_ph[:_upto])
        for rep in range(repeat):
            # ------------------------------------------------ phase A ----
            with (tc.tile_pool(name=f"pxa{rep}", bufs=1) as pxa,
                  tc.tile_pool(name=f"pa{rep}", bufs=3) as pa,
                  tc.tile_pool(name=f"pas{rep}", bufs=1) as pas,
                  tc.tile_pool(name=f"pah{rep}", bufs=1) as pah,
                  tc.tile_pool(name=f"paw{rep}", bufs=20) as paw,
                  tc.tile_pool(name=f"paps{rep}", bufs=3, space="PSUM") as paps,
                  tc.tile_pool(name=f"past{rep}", bufs=1, space="PSUM") as pstat):
                xTb = pxa.tile([P, CC, NT], f32r, name="xTb")
                for ci in range(CC):
                    (nc.sync if ci % 2 == 0 else nc.scalar).dma_start(
                        xTb[:, ci, :], xT_d[ci])
                xT = [xTb[:, ci, :] for ci in range(CC)]
                hb = pah.tile([P, CC, NT], f32r, name="hb")
                hts = layer_norm(xT, pa, pas, pstat, ln1s, ln1b,
                                 [hb[:, ci, :] for ci in range(CC)], "h", rep)
                for hc4 in range(4):
                    nc.sync.dma_start(
                        agin1[4 * hc4:4 * hc4 + 4].transpose([1, 0, 2]),
                        hb[:, 4 * hc4:4 * hc4 + 4, :])
                wd_tiles = []
                for ci in range(CC):
                    w = paw.tile([P, 8 * P], f32r, name="wdt", tag="wdt")
                    nc.gpsimd.dma_start(w[:], wdown_d[ci])
                    wd_tiles.append(w)
                dkvb = pah.tile([P, 8, NT], f32r, name="dkvb")
                for mi in range(8):
                    ps = paps.tile([P, NT], f32, name="psdkv", tag="psdkv")
                    for ci in range(CC):
                        nc.tensor.matmul(ps[:], wd_tiles[ci][:, mi * P:(mi + 1) * P],
                                         hts[ci],
                                         start=(ci == 0), stop=(ci == CC - 1))
                    nc.vector.tensor_scalar_add(dkvb[:, mi, :], ps[:],
                                                bdown[:, mi:mi + 1])
                nc.sync.dma_start(agin1[CC:20].transpose([1, 0, 2]),
                                  dkvb[:, 0:4, :])
                nc.sync.dma_start(agin1[20:24].transpose([1, 0, 2]),
                                  dkvb[:, 4:8, :])

            if nocc:
                nc.sync.dma_start(agout1h[0], agin1[0:CC])
                nc.sync.dma_start(agout1d[0], agin1[CC:24])
            else:
                nc.gpsimd.collective_compute(
                    "AllGather", mybir.AluOpType.bypass, replica_groups=RG,
                    ins=[agin1[0:CC].opt()], outs=[agout1h.opt()])
                nc.gpsimd.collective_compute(
                    "AllGather", mybir.AluOpType.bypass, replica_groups=RG,
                    ins=[agin1[CC:24].opt()], outs=[agout1d.opt()])

            # --------------------------------------------- phases B/C ----
            if "B1" not in _en:
                continue
            pprod_cm = tc.tile_pool(name=f"prod{rep}", bufs=1)
            pprod = pprod_cm.__enter__()
            qT4 = [pprod.tile([P, T], f32r, name=f"qT4_{m}") for m in range(4)]
            kT4 = [pprod.tile([P, T], f32r, name=f"kT4_{m}") for m in range(4)]
            # qR, kR projections -> DRAM spills; q/k stay in SBUF
            with (tc.tile_pool(name=f"pb1w{rep}", bufs=1) as pw,
                  tc.tile_pool(name=f"pb1s{rep}", bufs=20) as pstream,
                  tc.tile_pool(name=f"pb1c{rep}", bufs=6) as pcq,
                  tc.tile_pool(name=f"pb1t{rep}", bufs=3) as pt,
                  tc.tile_pool(name=f"pb1cs{rep}", bufs=3) as pcs,
                  tc.tile_pool(name=f"pb1ps{rep}", bufs=3, space="PSUM") as pps,
                  tc.tile_pool(name=f"pb1pr{rep}", bufs=2, space="PSUM") as ppsr):
                wqr_sb = []
                for ci in range(CC):
                    w = pw.tile([P, 2 * P], f32r, name=f"wqr{ci}")
                    nc.gpsimd.dma_start(w[:], wqr_d[ci])
                    wqr_sb.append(w)
                wkr_sb = []
                for ci in range(CC):
                    w = pw.tile([P, P], f32r, name=f"wkr{ci}")
                    nc.gpsimd.dma_start(w[:], wkr_d[ci])
                    wkr_sb.append(w)
                wuq_sb = []
                for lc in range(4):
                    w = pw.tile([P, 4 * P], f32r, name=f"wuq{lc}")
                    nc.gpsimd.dma_start(w[:], wuq_d[lc])
                    wuq_sb.append(w)

                def rope(pre, cos_t, sin_t, dst_ap):
                    rot = ppsr.tile([P, NT], f32, name="psrot", tag="psrot")
                    nc.tensor.matmul(rot[:], r2[:], pre[:], start=True, stop=True)
                    tmp = pt.tile([P, NT], f32, name="rtmp", tag="rtmp")
                    nc.vector.tensor_mul(tmp[:], rot[:], sin_t[:])
                    tmp2 = pt.tile([P, NT], f32, name="rtmp2", tag="rtmp2")
                    nc.vector.tensor_mul(tmp2[:], pre[:], cos_t[:])
                    out = pt.tile([P, NT], f32r, name="rout", tag="rout")
                    nc.vector.tensor_add(out[:], tmp2[:], tmp[:])
                    nc.sync.dma_start(dst_ap, out[:])

                for nt in range(4):
                    nts = slice(nt * NT, (nt + 1) * NT)
                    ht = []
                    for ci in range(CC):
                        t = pstream.tile([P, NT], f32r, name="htc", tag="htc")
                        (nc.sync if ci % 2 == 0 else nc.scalar).dma_start(
                            t[:], agout1h[nt, ci])
                        ht.append(t)
                    cq = []
                    for lc in range(4):
                        t = pcq.tile([P, NT], f32r, name="cqc", tag="cqc")
                        nc.sync.dma_start(t[:], agout1d[nt, 4 + lc])
                        cq.append(t)
                    for mt in range(2):
                        ps = pps.tile([P, NT], f32, name="psqr", tag="psqr")
                        for ci in range(CC):
                            nc.tensor.matmul(
                                ps[:], wqr_sb[ci][:, mt * P:(mt + 1) * P],
                                ht[ci], start=(ci == 0), stop=(ci == CC - 1))
                        pre = pt.tile([P, NT], f32r, name="qrpre", tag="qrpre")
                        nc.scalar.activation(pre[:], ps[:], AF.Identity,
                                             bias=bqr[:, mt:mt + 1])
                        cos_t = pcs.tile([P, NT], f32, name="cosq", tag="cosq")
                        nc.scalar.dma_start(cos_t[:], cosq_d[mt, nt])
                        sin_t = pcs.tile([P, NT], f32, name="sinq", tag="sinq")
                        nc.scalar.dma_start(sin_t[:], sinq_d[mt, nt])
                        rope(pre, cos_t, sin_t, qR_sp[mt][:, nts])
                    # kR
                    ps = pps.tile([P, NT], f32, name="pskr", tag="psqr")
                    for ci in range(CC):
                        nc.tensor.matmul(ps[:], wkr_sb[ci][:], ht[ci][:],
                                         start=(ci == 0), stop=(ci == CC - 1))
                    pre = pt.tile([P, NT], f32r, name="krpre", tag="qrpre")
                    nc.scalar.activation(pre[:], ps[:], AF.Identity,
                                         bias=bkr[:, 0:1])
                    cos_t = pcs.tile([P, NT], f32, name="cosk", tag="cosq")
                    nc.scalar.dma_start(cos_t[:], cosk_d[nt])
                    sin_t = pcs.tile([P, NT], f32, name="sink", tag="sinq")
                    nc.scalar.dma_start(sin_t[:], sink_d[nt])
                    rope(pre, cos_t, sin_t, kR_sp[:, nts])
                    # q
                    for mt in range(4):
                        ps = pps.tile([P, NT], f32, name="psq", tag="psqr")
                        for lc in range(4):
                            nc.tensor.matmul(
                                ps[:], wuq_sb[lc][:, mt * P:(mt + 1) * P],
                                cq[lc], start=(lc == 0), stop=(lc == 3))
                        nc.vector.tensor_scalar_add(qT4[mt][:, nts], ps[:],
                                                    buq[:, mt:mt + 1])

            # --------------------------------------------- phase B2 ----
            if "B2" not in _en:
                pprod_cm.__exit__(None, None, None)
                continue
            with (tc.tile_pool(name=f"pb2w{rep}", bufs=1) as pw2,
                  tc.tile_pool(name=f"pb2s{rep}", bufs=6) as pkv,
                  tc.tile_pool(name=f"pb2t{rep}", bufs=3) as pt2,
                  tc.tile_pool(name=f"pb2ps{rep}", bufs=3, space="PSUM") as pps2):
                wuk_sb = []
                wuv_sb = []
                for lc in range(4):
                    w = pw2.tile([P, 4 * P], f32r, name=f"wuk{lc}")
                    nc.gpsimd.dma_start(w[:], wuk_d[lc])
                    wuk_sb.append(w)
                    w = pw2.tile([P, 4 * P], f32r, name=f"wuv{lc}")
                    nc.gpsimd.dma_start(w[:], wuv_d[lc])
                    wuv_sb.append(w)
                for nt in range(4):
                    nts = slice(nt * NT, (nt + 1) * NT)
                    ckv = []
                    for lc in range(4):
                        t = pkv.tile([P, NT], f32r, name="ckvc", tag="ckvc")
                        nc.sync.dma_start(t[:], agout1d[nt, lc])
                        ckv.append(t)
                    for mt in range(4):
                        ps = pps2.tile([P, NT], f32, name="psk", tag="psk")
                        for lc in range(4):
                            nc.tensor.matmul(
                                ps[:], wuk_sb[lc][:, mt * P:(mt + 1) * P],
                                ckv[lc], start=(lc == 0), stop=(lc == 3))
                        nc.vector.tensor_scalar_add(kT4[mt][:, nts], ps[:],
                                                    buk[:, mt:mt + 1])
                    for tt in range(4):
                        ps = pps2.tile([P, 4 * P], f32, name="psv", tag="psk")
                        for lc in range(4):
                            nc.tensor.matmul(
                                ps[:], ckv[lc][:, tt * P:(tt + 1) * P],
                                wuv_sb[lc][:], start=(lc == 0), stop=(lc == 3))
                        vo = pt2.tile([P, 4 * P], f32r, name="vout", tag="vout")
                        nc.vector.tensor_copy(vo[:], ps[:])
                        (nc.sync if tt % 2 == 0 else nc.scalar).dma_start(
                            v_sp[4 * nt + tt], vo[:])

            # ---------------------------------------------- phase C ----
            if "C" not in _en:
                pprod_cm.__exit__(None, None, None)
                continue
            with (tc.tile_pool(name=f"pch{rep}", bufs=2) as phd,
                  tc.tile_pool(name=f"pcm{rep}", bufs=1) as pcm,
                  tc.tile_pool(name=f"pce{rep}", bufs=6) as pex,
                  tc.tile_pool(name=f"pco{rep}", bufs=3) as pot,
                  tc.tile_pool(name=f"pcps{rep}", bufs=4, space="PSUM") as pcsc,
                  tc.tile_pool(name=f"pcpo{rep}", bufs=2, space="PSUM") as pcso,
                  tc.tile_pool(name=f"pcpm{rep}", bufs=2, space="PSUM") as pcss):
                masks = []
                for j in range(4):
                    m_ = pcm.tile([P, NT], f32, name=f"mask{j}")
                    nc.gpsimd.dma_start(m_[:], mask_d[j])
                    masks.append(m_)
                for h in range(4):
                    kTh = kT4[h]
                    qTh = qT4[h]
                    qRh = phd.tile([DHR, T], f32r, name="qRh", tag="qRh")
                    nc.sync.dma_start(
                        qRh[:], qR_sp[h // 2][DHR * (h % 2):DHR * (h % 2) + DHR, :])
                    kRh = phd.tile([DHR, T], f32r, name="kRh", tag="kRh")
                    nc.sync.dma_start(kRh[:], kR_sp[0:DHR, :])
                    vh = phd.tile([P, 16, P], f32r, name="vh", tag="vh")
                    # one strided read: DRAM [16, P, P-slice] iterated (p, tt, f)
                    vsrc = v_sp[:, :, h * P:(h + 1) * P].transpose([1, 0, 2])
                    nc.sync.dma_start(vh[:], vsrc)
                    for qi in range(4):
                        qs = slice(qi * NT, (qi + 1) * NT)
                        pso = pcso.tile([P, NT], f32, name="pso", tag="pso")
                        pss = pcss.tile([P, NT], f32, name="pss", tag="pss")
                        nki = 4 * qi + 4
                        for ki in range(nki):
                            ks = slice(ki * P, (ki + 1) * P)
                            psc = pcsc.tile([P, NT], f32, name="psc", tag="psc")
                            nc.tensor.matmul(psc[:], kTh[:, ks], qTh[:, qs],
                                             start=True, stop=False)
                            nc.tensor.matmul(psc[:], kRh[:, ks], qRh[:, qs],
                                             start=False, stop=True)
                            if ki >= 4 * qi:
                                nc.vector.tensor_add(psc[:], psc[:],
                                                     masks[ki - 4 * qi][:])
                            ex = pex.tile([P, NT], f32r, name="ex", tag="ex")
                            nc.scalar.activation(ex[:], psc[:], AF.Exp,
                                                 scale=SCALE)
                            nc.tensor.matmul(pso[:], vh[:, ki, :], ex[:],
                                             start=(ki == 0),
                                             stop=(ki == nki - 1))
                            nc.tensor.matmul(pss[:], ones_r[:], ex[:],
                                             start=(ki == 0),
                                             stop=(ki == nki - 1))
                        rec = pot.tile([P, NT], f32, name="rec", tag="rec")
                        nc.vector.reciprocal(rec[:], pss[:])
                        ot = pot.tile([P, NT], f32, name="ot", tag="ot")
                        nc.vector.tensor_mul(ot[:], pso[:], rec[:])
                        otb = pot.tile([P, NT], f32r, name="otb", tag="otb")
                        nc.vector.tensor_scalar_add(otb[:], ot[:],
                                                    buv[:, h:h + 1])
                        nc.sync.dma_start(agin2[h][:, qs], otb[:])

            pprod_cm.__exit__(None, None, None)
            if nocc:
                nc.sync.dma_start(agout2[0:4], agin2[:])
            else:
                nc.gpsimd.collective_compute(
                    "AllGather", mybir.AluOpType.bypass, replica_groups=RG,
                    ins=[agin2.opt()], outs=[agout2.opt()])

            # ------------------------------------------------ phase D ----
            if "D" not in _en:
                continue
            with tc.tile_pool(name=f"pde{rep}", bufs=1) as pper:
                with (tc.tile_pool(name=f"pxd{rep}", bufs=1) as pxd,
                      tc.tile_pool(name=f"pdo{rep}", bufs=1) as pdo,
                      tc.tile_pool(name=f"pdw{rep}", bufs=6) as pdw,
                      tc.tile_pool(name=f"pdt{rep}", bufs=3) as pdt,
                      tc.tile_pool(name=f"pds{rep}", bufs=1) as pds,
                      tc.tile_pool(name=f"pdps{rep}", bufs=3, space="PSUM") as pdps,
                      tc.tile_pool(name=f"pdst{rep}", bufs=1, space="PSUM") as pdst):
                    xT2 = []
                    for ci in range(CC):
                        t = pxd.tile([P, NT], f32r, name=f"xTd{ci}")
                        nc.sync.dma_start(t[:], xT_d[ci])
                        xT2.append(t)
                    otb_ = pdo.tile([P, 16, NT], f32r, name="otb_")
                    nc.sync.dma_start(
                        otb_[:],
                        agout2[:, :, bass.ds(colo, NT)].transpose([1, 0, 2]))
                    otsl = [otb_[:, oi, :] for oi in range(16)]
                    xmid = []
                    for mig in range(4):
                        wots = []
                        for kg in range(4):
                            wg = pdw.tile([P, 4, 4 * P], f32r, name="wog",
                                          tag="wog")
                            nc.sync.dma_start(
                                wg[:], wo_d[4 * kg:4 * kg + 4, mig]
                                .transpose([1, 0, 2]))
                            for kl in range(4):
                                wots.append(wg[:, kl, :])
                        for ml in range(4):
                            mi = mig * 4 + ml
                            ps = pdps.tile([P, NT], f32, name="pswo", tag="pswo")
                            for ki in range(16):
                                nc.tensor.matmul(
                                    ps[:], wots[ki][:, ml * P:(ml + 1) * P],
                                    otsl[ki], start=(ki == 0), stop=(ki == 15))
                            xm = pper.tile([P, NT], f32r, name=f"xmid{mi}")
                            nc.vector.scalar_tensor_tensor(
                                xm[:], ps[:], bo[:, mi:mi + 1], xT2[mi][:],
                                OP.add, OP.add)
                            xmid.append(xm)
                    h2_tiles = [pper.tile([P, NT], f32r, name=f"h2_{ci}")
                                for ci in range(CC)]
                    h2 = layer_norm([t[:] for t in xmid], pdt, pds, pdst, ln2s, ln2b,
                                    [t[:] for t in h2_tiles], "h2_", rep)

                # -------------------------------------------- phase E ----
                if "E" not in _en:
                    continue
                with (tc.tile_pool(name=f"pew{rep}", bufs=17) as pew,
                      tc.tile_pool(name=f"pew2{rep}", bufs=2) as pew2,
                      tc.tile_pool(name=f"peg{rep}", bufs=17) as peg,
                      tc.tile_pool(name=f"pea{rep}", bufs=1) as pea,
                      tc.tile_pool(name=f"pet{rep}", bufs=3) as pet,
                      tc.tile_pool(name=f"peps{rep}", bufs=3, space="PSUM") as peps,
                      tc.tile_pool(name=f"pep2{rep}", bufs=2, space="PSUM") as pep2):
                    accs = [pea.tile([P, NT], f32, name=f"ffacc{mi}")
                            for mi in range(16)]
                    for hb in range(4):
                        gts = []
                        for mtg in range(4):
                            mtg_g = hb * 4 + mtg
                            wts = []
                            for ci in range(CC):
                                w = pew.tile([P, 4 * P], f32r, name="wf1",
                                             tag="wf1")
                                nc.sync.dma_start(w[:], wff1_d[ci, mtg_g])
                                wts.append(w)
                            for ml in range(4):
                                mt = mtg_g * 4 + ml
                                ps = peps.tile([P, NT], f32, name="psf1",
                                               tag="psf1")
                                for ci in range(CC):
                                    nc.tensor.matmul(
                                        ps[:], wts[ci][:, ml * P:(ml + 1) * P],
                                        h2[ci],
                                        start=(ci == 0), stop=(ci == CC - 1))
                                gt = peg.tile([P, NT], f32r, name="gt", tag="gt")
                                nc.scalar.activation(gt[:], ps[:],
                                                     AF.Gelu_apprx_tanh,
                                                     bias=bff1[:, mt:mt + 1])
                                gts.append(gt)
                        for mi in range(16):
                            w2 = pew2.tile([P, CC * P], f32r, name="wf2",
                                           tag="wf2")
                            nc.sync.dma_start(w2[:], wff2_d[hb, mi])
                            ps2 = pep2.tile([P, NT], f32, name="psf2",
                                            tag="psf2")
                            for hl in range(16):
                                nc.tensor.matmul(
                                    ps2[:], w2[:, hl * P:(hl + 1) * P],
                                    gts[hl][:],
                                    start=(hl == 0), stop=(hl == 15))
                            if hb == 0:
                                nc.vector.tensor_copy(accs[mi][:], ps2[:])
                            else:
                                nc.vector.tensor_add(accs[mi][:], accs[mi][:],
                                                     ps2[:])
                    for mi in range(CC):
                        ob = pet.tile([P, NT], f32, name="outb", tag="outb")
                        nc.vector.scalar_tensor_tensor(
                            ob[:], accs[mi][:], bff2[:, mi:mi + 1],
                            xmid[mi][:], OP.add, OP.add)
                        nc.sync.dma_start(outT_d[mi], ob[:])

    nc.compile()
    return nc


# ------------------------------------------------------------------ host ---
def _rope_tables(r):
    """cos/sin tiles for core rank r (heads 4r..4r+3)."""
    t = np.arange(T, dtype=np.float64) + 1.0
    l = np.arange(DHR)
    cosq = np.zeros((2, P, T), np.float64)
    sinq = np.zeros((2, P, T), np.float64)
    for mt in range(2):
        for hl in range(2):
            h = 4 * r + 2 * mt + hl
            theta = 10000.0 ** (-2.0 * (32 * h + l // 2) / 1024.0)
            ang = t[None, :] * theta[:, None]            # [64, T]
            cosq[mt, 64 * hl:64 * hl + 64] = np.cos(ang)
            sinq[mt, 64 * hl:64 * hl + 64] = np.sin(ang)
    thk = 10000.0 ** (-2.0 * (l // 2) / 64.0)
    angk = t[None, :] * thk[:, None]
    cosk = np.concatenate([np.cos(angk)] * 2, axis=0)     # [128, T]
    sink = np.concatenate([np.sin(angk)] * 2, axis=0)
    cosq = cosq.reshape(2, P, 4, NT).transpose(0, 2, 1, 3)
    sinq = sinq.reshape(2, P, 4, NT).transpose(0, 2, 1, 3)
    cosk = cosk.reshape(P, 4, NT).transpose(1, 0, 2)
    sink = sink.reshape(P, 4, NT).transpose(1, 0, 2)
    f = np.float32
    return (np.ascontiguousarray(cosq, f), np.ascontiguousarray(sinq, f),
            np.ascontiguousarray(cosk, f), np.ascontiguousarray(sink, f))


def _shared_consts():
    r2 = np.zeros((P, P), np.float32)
    for i in range(64):
        r2[2 * i + 1, 2 * i] = -1.0
        r2[2 * i, 2 * i + 1] = 1.0
    mask = np.zeros((4, P, NT), np.float32)
    kl = np.arange(P)[:, None]
    ql = np.arange(NT)[None, :]
    for j in range(4):
        mask[j] = np.where(P * j + kl > ql, NEG, 0.0)
    ones = np.ones((P, P), np.float32)
    return r2, mask, ones


def prepare_in_maps(inputs):
    f = np.float32
    g = {k: np.asarray(v, f) for k, v in inputs.items()}
    x = g["x"]
    r2, mask, ones = _shared_consts()

    wdown_t = np.ascontiguousarray(g["w_down"].reshape(CC, P, 8 * P))
    bdown_t = np.ascontiguousarray(g["b_down"].reshape(8, P).T)
    wkr2 = np.concatenate([g["w_kr"], g["w_kr"]], axis=1)  # [C, 128]
    wkr_t = np.ascontiguousarray(wkr2.reshape(CC, P, P))
    bkr_t = np.ascontiguousarray(
        np.concatenate([g["b_kr"], g["b_kr"]]).reshape(P, 1))
    wo_t = np.ascontiguousarray(
        g["w_o"].reshape(CC, P, 4, 4 * P).transpose(0, 2, 1, 3))
    bo_t = np.ascontiguousarray(g["b_o"].reshape(CC, P).T)
    wff1_t = np.ascontiguousarray(
        g["w_ff1"].reshape(CC, P, 16, 4 * P).transpose(0, 2, 1, 3))
    bff1_t = np.ascontiguousarray(g["b_ff1"].reshape(64, P).T)
    wff2_t = np.ascontiguousarray(
        g["w_ff2"].reshape(4, CC, P, CC, P).transpose(0, 3, 2, 1, 4)
        .reshape(4, CC, P, CC * P))
    bff2_t = np.ascontiguousarray(g["b_ff2"].reshape(CC, P).T)
    ln1s_t = np.ascontiguousarray(g["ln1_scale"].reshape(CC, P).T)
    ln1b_t = np.ascontiguousarray(g["ln1_bias"].reshape(CC, P).T)
    ln2s_t = np.ascontiguousarray(g["ln2_scale"].reshape(CC, P).T)
    ln2b_t = np.ascontiguousarray(g["ln2_bias"].reshape(CC, P).T)

    in_maps = []
    for c in range(NCORES):
        b, r = divmod(c, 4)
        cosq, sinq, cosk, sink = _rope_tables(r)
        xs = x[b, NT * r:NT * (r + 1), :].T                      # [C, NT]
        xT_t = np.ascontiguousarray(xs.reshape(CC, P, NT))
        wqr_c = g["w_qr"][:, 256 * r:256 * (r + 1)]
        wuk_c = g["w_ukv"][:, 512 * r:512 * (r + 1)]
        wuv_c = g["w_ukv"][:, C + 512 * r:C + 512 * (r + 1)]
        wuq_c = g["w_uq"][:, 512 * r:512 * (r + 1)]
        m = {
            "xT": xT_t,
            "ln1s": ln1s_t, "ln1b": ln1b_t, "ln2s": ln2s_t, "ln2b": ln2b_t,
            "wdown": wdown_t, "bdown": bdown_t,
            "wqr": np.ascontiguousarray(wqr_c.reshape(CC, P, 2 * P)),
            "bqr": np.ascontiguousarray(
                g["b_qr"][256 * r:256 * (r + 1)].reshape(2, P).T),
            "wkr": wkr_t, "bkr": bkr_t,
            "r2": r2,
            "cosq": cosq, "sinq": sinq, "cosk": cosk, "sink": sink,
            "wuk": np.ascontiguousarray(wuk_c.reshape(4, P, 4 * P)),
            "buk": np.ascontiguousarray(
                g["b_ukv"][512 * r:512 * (r + 1)].reshape(4, P).T),
            "wuv": np.ascontiguousarray(wuv_c.reshape(4, P, 4 * P)),
            "buv": np.ascontiguousarray(
                g["b_ukv"][C + 512 * r:C + 512 * (r + 1)].reshape(4, P).T),
            "wuq": np.ascontiguousarray(wuq_c.reshape(4, P, 4 * P)),
            "buq": np.ascontiguousarray(
                g["b_uq"][512 * r:512 * (r + 1)].reshape(4, P).T),
            "mask": mask, "ones_r": ones,
            "wo": wo_t, "bo": bo_t,
            "wff1": wff1_t, "bff1": bff1_t,
            "wff2": wff2_t, "bff2": bff2_t,
        }
        in_maps.append(m)
    return in_maps


def assemble_output(results):
    out = np.zeros((B, T, C), np.float32)
    for c in range(NCORES):
        b, r = divmod(c, 4)
        o = results[c]["outT"].reshape(C, NT)
        out[b, NT * r:NT * (r + 1), :] = o.T
    return out


def kernel(**inputs):
    from concourse import bass_utils
    nc = _CACHE.get("nc")
    if nc is None:
        nc = build_program(repeat=1)
        _CACHE["nc"] = nc
    in_maps = prepare_in_maps(inputs)
    res = bass_utils.run_bass_kernel_spmd(nc, in_maps,
                                          core_ids=list(range(NCORES)))
    return assemble_output(res.results)



# revision 23
# speedup vs baseline: 8.9540x; 8.9540x over previous
"""Trainium2 Bass kernel for nn_Block_9268539425531 (MLA transformer block).

v4: 2 batch groups x 4-way head-TP within each group of 4 cores.
Per core (b = core//4, r = core%4, heads H = [4r, 4r+4)):
  Host folds w_down@w_ukv / w_down@w_uq (and the LN1/LN2 affine) into
  per-core effective fp16 weights, so each core computes LN1 for the full
  2048 tokens cheaply (matmul-trick stats, in-place normalize) and
  projects q/k/v/qR/kR for its own 4 heads directly -- no pre-attention
  collective.
  kR's 64-dim decoupled-RoPE uses two zero-padded weight variants
  (even/odd head parity) so score matmuls need no partition shifts.
  The v bias is applied post-softmax (softmax rows sum to 1).
  Attention runs head-outer; per-head outputs are AllGather'd in two fp16
  chunks (head pairs), overlapped with the remaining heads' compute and
  with the first half of the w_o matmul (two accumulation passes).
  FFN is token-sharded (512 tokens/core, full hidden dim).
All matmul operands fp16 (full rate on TRN2), accumulation fp32.
All bulk DRAM tensors are host-pre-transposed to partition-major layouts
so every DMA moves long contiguous runs; scalar constants ride in two
packed tensors (one f32, one f16).
"""
import math
import numpy as np

B, T, C = 2, 2048, 2048
NH = 16
DK = 128
DHR = 64
LAT = 512
P = 128
NT = 512           # tokens per core
CC = C // P        # 16
NCORES = 8
SCALE = 1.0 / math.sqrt(DK)
NEG = -1.0e9
RG = [[0, 1, 2, 3], [4, 5, 6, 7]]

_CACHE = {}


# ---------------------------------------------------------------- program ---
def build_program(repeat=1, nocc=False, dbg=False):
    from contextlib import ExitStack
    from concourse import bass, bacc, tile, mybir

    dt = mybir.dt
    f32 = dt.float32
    f16 = dt.float16
    AF = mybir.ActivationFunctionType
    OP = mybir.AluOpType

    nc = bacc.Bacc("TRN2", target_bir_lowering=False, debug=False,
                   num_devices=NCORES)

    def din(name, shape, dtype=f16):
        return nc.dram_tensor(name, shape, dtype, kind="ExternalInput")

    xbf_d = din("xbf", [4, P, CC, NT])                # full x by token block
    xown_d = din("xown16", [P, CC, NT])               # own block, fp16
    cf32_d = din("cf32", [P, 112], f32)               # packed f32 consts
    cf16_d = din("cf16", [P, 2 * P])                  # ones | r2
    mask_d = din("mask", [P, 4, NT], f32)
    wq_d = din("wq", [P, CC, 4 * P])
    wk_d = din("wk", [P, CC, 4 * P])
    wv_d = din("wv", [P, CC, 4 * P])
    wqr_d = din("wqr", [P, CC, 2 * P])
    wkr_d = din("wkr", [2, P, CC, P])                 # even/odd zero-padded
    trig_d = din("trig", [4, P, 6, NT])               # cq0 cq1 sq0 sq1 ck sk
    wo_d = din("wo", [4, P, CC, 4 * P])               # [mig, P, ki, 512]
    wff1_d = din("wff1", [16, P, CC, 4 * P])          # [mtg, P, ci, 512]
    wff2_d = din("wff2", [4, CC, P, CC * P])          # [hb, mi, P, 2048]
    outT_d = nc.dram_tensor("outT", [CC, P, NT], f32, kind="ExternalOutput")

    dbg_d = {}
    if dbg:
        for nm, shape in [("d_qT", [4, P, T]), ("d_kT", [4, P, T]),
                          ("d_vS", [P, 16, 4 * P]), ("d_qR", [2, P, T]),
                          ("d_kR", [2, P, T]), ("d_o", [4, P, T]),
                          ("d_xmid", [CC, P, NT]), ("d_h2", [CC, P, NT]),
                          ("d_h1", [CC, P, T])]:
            dbg_d[nm] = nc.dram_tensor(nm, shape, f32, kind="ExternalOutput")

    with tile.TileContext(nc) as tc, ExitStack() as ctx:
        pc = ctx.enter_context(tc.tile_pool(name="const", bufs=1))
        pdram = ctx.enter_context(tc.tile_pool(name="dram", bufs=1, space="DRAM"))

        cb = pc.tile([P, 112], f32, name="cb")
        nc.sync.dma_start(cb[:], cf32_d[:])
        c16 = pc.tile([P, 2 * P], f16, name="c16")
        nc.sync.dma_start(c16[:], cf16_d[:])
        bq = cb[:, 0:4]
        bk = cb[:, 4:8]
        bv = cb[:, 8:12]
        bqr = cb[:, 12:14]
        bkrD = cb[:, 14:15]
        bo = cb[:, 16:32]
        bff1 = cb[:, 32:96]
        bff2 = cb[:, 96:112]
        ones_r = c16[:, 0:P]
        r2 = c16[:, P:2 * P]
        eps_t = pc.tile([P, 1], f32, name="eps_t")
        nc.vector.memset(eps_t[:], 1e-6)

        o_sp = pdram.tile([4, P, T], f16, name="o_sp")
        ogh = [pdram.tile([4, P, T], f16, name=f"ogh{h}") for h in range(4)]

        pid = nc.sync.partition_id()
        colo = (pid % 4) * NT

        for rep in range(repeat):
            sfx = f"r{rep}"
            # ogt0 staged early so the first AllGather's output can stream
            # into SBUF while heads 2-3 still compute
            pm0_cm = tc.tile_pool(name=f"pm0{sfx}", bufs=1)
            pm0 = pm0_cm.__enter__()
            ogt0 = pm0.tile([P, 4, 2, NT], f16, name="ogt0")
            woT01 = pm0.tile([P, CC, 4 * P], f16, name="woT01")
            # persistent across AB + C
            pprod_cm = tc.tile_pool(name=f"prod{sfx}", bufs=1)
            pprod = pprod_cm.__enter__()
            qT = [pprod.tile([P, T], f16, name=f"qT{m}") for m in range(4)]
            kT = [pprod.tile([P, T], f16, name=f"kT{m}") for m in range(4)]
            vS = pprod.tile([P, 16, 4 * P], f16, name="vS")
            qRt = [pprod.tile([P, T], f16, name=f"qR{m}") for m in range(2)]
            kRt = [pprod.tile([P, T], f16, name=f"kR{m}") for m in range(2)]
            nc.vector.memset(kRt[0][DHR:P, :], 0.0)
            nc.vector.memset(kRt[1][0:DHR, :], 0.0)

            # ------------------------------------------------ phase AB ----
            with (tc.tile_pool(name=f"pabw{sfx}", bufs=1) as pw,
                  tc.tile_pool(name=f"pabx{sfx}", bufs=2) as px,
                  tc.tile_pool(name=f"pabsq{sfx}", bufs=2) as psq,
                  tc.tile_pool(name=f"pabcs{sfx}", bufs=1) as pcs,
                  tc.tile_pool(name=f"pabt{sfx}", bufs=2) as pt,
                  tc.tile_pool(name=f"pabps{sfx}", bufs=2, space="PSUM") as pps,
                  tc.tile_pool(name=f"pabpp{sfx}", bufs=2, space="PSUM") as ppp):
                wq_sb = wk_sb = wv_sb = wqr_sb = wkrD_sb = None

                def load_weights():
                    nonlocal wq_sb, wk_sb, wv_sb, wqr_sb, wkrD_sb
                    wqT = pw.tile([P, CC, 4 * P], f16, name="wqT")
                    nc.sync.dma_start(wqT[:], wq_d[:])
                    wkT = pw.tile([P, CC, 4 * P], f16, name="wkT")
                    nc.sync.dma_start(wkT[:], wk_d[:])
                    wvT = pw.tile([P, CC, 4 * P], f16, name="wvT")
                    nc.sync.dma_start(wvT[:], wv_d[:])
                    wrT = pw.tile([P, CC, 2 * P], f16, name="wrT")
                    nc.sync.dma_start(wrT[:], wqr_d[:])
                    wkdT = pw.tile([P, CC, P], f16, name="wkdT")
                    nc.sync.dma_start(wkdT[:], wkr_d[0])
                    wq_sb = [wqT[:, ci, :] for ci in range(CC)]
                    wk_sb = [wkT[:, ci, :] for ci in range(CC)]
                    wv_sb = [wvT[:, ci, :] for ci in range(CC)]
                    wqr_sb = [wrT[:, ci, :] for ci in range(CC)]
                    wkrD_sb = [wkdT[:, ci, :] for ci in range(CC)]
                    nc.sync.dma_start(woT01[:], wo_d[0])

                def rope(pre, cos_t, sin_t, dst_ap):
                    rot = ppp.tile([P, NT], f32, name="psrot", tag="psrot",
                                   bufs=1)
                    nc.tensor.matmul(rot[:], r2, pre[:], start=True,
                                     stop=True)
                    tmp = pt.tile([P, NT], f16, name="rtmp", tag="rtmp",
                                  bufs=1)
                    nc.vector.tensor_mul(tmp[:], rot[:], sin_t)
                    tmp2 = pt.tile([P, NT], f16, name="rtmp2", tag="rtmp2",
                                   bufs=1)
                    nc.vector.tensor_mul(tmp2[:], pre[:], cos_t)
                    nc.vector.tensor_add(dst_ap, tmp2[:], tmp[:])

                hts = [None] * 4   # per-nt normalized-x tiles (in-place)

                def emit_stats(nt):
                    xb = px.tile([P, CC, NT], f16, name="xb", tag="xb")
                    nc.sync.dma_start(xb[:, 0:8, :], xbf_d[nt, :, 0:8, :])
                    nc.sync.dma_start(xb[:, 8:CC, :], xbf_d[nt, :, 8:CC, :])
                    ps_mean = pps.tile([P, NT], f32, name="psm", tag="psm")
                    ps_sq = pps.tile([P, NT], f32, name="pss", tag="pss")
                    for ci in range(CC):
                        sq = psq.tile([P, NT], f16, name="sq", tag="sq")
                        if ci % 2 == 0:
                            nc.scalar.square(sq[:], xb[:, ci, :])
                        else:
                            nc.gpsimd.tensor_mul(sq[:], xb[:, ci, :],
                                                 xb[:, ci, :])
                        nc.tensor.matmul(ps_mean[:], ones_r, xb[:, ci, :],
                                         start=(ci == 0), stop=(ci == CC - 1),
                                         skip_group_check=True)
                        nc.tensor.matmul(ps_sq[:], ones_r, sq[:],
                                         start=(ci == 0), stop=(ci == CC - 1),
                                         skip_group_check=True)
                    meanb = pt.tile([P, NT], f32, name="meanb", tag="meanb")
                    nc.vector.tensor_scalar_mul(meanb[:], ps_mean[:], 1.0 / C)
                    m2 = pt.tile([P, NT], f32, name="m2", tag="m2", bufs=1)
                    nc.vector.tensor_mul(m2[:], meanb[:], meanb[:])
                    var = pt.tile([P, NT], f32, name="var", tag="var",
                                  bufs=1)
                    nc.vector.scalar_tensor_tensor(var[:], ps_sq[:], 1.0 / C,
                                                   m2[:], OP.mult, OP.subtract)
                    std = pt.tile([P, NT], f32, name="std", tag="m2",
                                  bufs=1)
                    nc.scalar.activation(std[:], var[:], AF.Sqrt,
                                         bias=eps_t[:])
                    rstd = pt.tile([P, NT], f32, name="rstd", tag="rstd")
                    nc.vector.reciprocal(rstd[:], std[:])
                    # normalize in place: xb <- (xb - mean) * rstd
                    for ci in range(CC):
                        eng = nc.gpsimd if ci % 4 == 3 else nc.vector
                        t1 = psq.tile([P, NT], f16, name="t1", tag="t1")
                        eng.tensor_sub(t1[:], xb[:, ci, :], meanb[:])
                        eng.tensor_mul(xb[:, ci, :], t1[:], rstd[:])
                    hts[nt] = xb

                def emit_proj(nt):
                    nts = slice(nt * NT, (nt + 1) * NT)
                    hb = hts[nt]
                    h = [hb[:, ci, :] for ci in range(CC)]
                    trig = pcs.tile([P, 6, NT], f16, name="trig", tag="trig")
                    nc.scalar.dma_start(trig[:], trig_d[nt])
                    if dbg:
                        for ci in range(CC):
                            hf = pt.tile([P, NT], f32, name="hdump",
                                         tag="hdump", bufs=2)
                            nc.vector.tensor_copy(hf[:], h[ci])
                            nc.sync.dma_start(dbg_d["d_h1"][ci, :, nts], hf[:])
                    for m in range(4):
                        ps = ppp.tile([P, NT], f32, name="psp", tag="psp")
                        for ci in range(CC):
                            nc.tensor.matmul(ps[:],
                                             wq_sb[ci][:, m * P:(m + 1) * P],
                                             h[ci], start=(ci == 0),
                                             stop=(ci == CC - 1))
                        nc.scalar.activation(qT[m][:, nts], ps[:], AF.Identity,
                                             bias=bq[:, m:m + 1])
                    for m in range(4):
                        ps = ppp.tile([P, NT], f32, name="psp", tag="psp")
                        for ci in range(CC):
                            nc.tensor.matmul(ps[:],
                                             wk_sb[ci][:, m * P:(m + 1) * P],
                                             h[ci], start=(ci == 0),
                                             stop=(ci == CC - 1))
                        nc.scalar.activation(kT[m][:, nts], ps[:], AF.Identity,
                                             bias=bk[:, m:m + 1])
                    for tt in range(4):
                        ps = ppp.tile([P, 4 * P], f32, name="psp", tag="psp")
                        for ci in range(CC):
                            nc.tensor.matmul(ps[:],
                                             h[ci][:, tt * P:(tt + 1) * P],
                                             wv_sb[ci], start=(ci == 0),
                                             stop=(ci == CC - 1))
                        nc.scalar.activation(vS[:, 4 * nt + tt, :], ps[:],
                                             AF.Identity)
                    # qR (2 chunks = 4 heads, 64-dim pairs on partitions)
                    for mt in range(2):
                        ps = ppp.tile([P, NT], f32, name="psp", tag="psp")
                        for ci in range(CC):
                            nc.tensor.matmul(ps[:],
                                             wqr_sb[ci][:, mt * P:(mt + 1) * P],
                                             h[ci], start=(ci == 0),
                                             stop=(ci == CC - 1))
                        pre = pt.tile([P, NT], f16, name="pre", tag="pre",
                                      bufs=1)
                        nc.scalar.activation(pre[:], ps[:], AF.Identity,
                                             bias=bqr[:, mt:mt + 1])
                        rope(pre, trig[:, mt, :], trig[:, 2 + mt, :],
                             qRt[mt][:, nts])
                    # kR: one duplicated-layout chain; the per-parity
                    # zero halves are static (memset once per rep)
                    ps = ppp.tile([P, NT], f32, name="psp", tag="psp")
                    for ci in range(CC):
                        nc.tensor.matmul(ps[:], wkrD_sb[ci], h[ci],
                                         start=(ci == 0),
                                         stop=(ci == CC - 1))
                    pre = pt.tile([P, NT], f16, name="pre", tag="pre",
                                  bufs=1)
                    nc.scalar.activation(pre[:], ps[:], AF.Identity,
                                         bias=bkrD)
                    rot = ppp.tile([P, NT], f32, name="psrot", tag="psrot",
                                   bufs=1)
                    nc.tensor.matmul(rot[:], r2, pre[:], start=True, stop=True)
                    tmp = pt.tile([P, NT], f16, name="rtmp", tag="rtmp",
                                  bufs=1)
                    nc.vector.tensor_mul(tmp[:], rot[:], trig[:, 5, :])
                    tmp2 = pt.tile([P, NT], f16, name="rtmp2", tag="rtmp2",
                                   bufs=1)
                    nc.vector.tensor_mul(tmp2[:], pre[:], trig[:, 4, :])
                    nc.vector.tensor_add(kRt[0][0:DHR, nts], tmp2[0:DHR, :],
                                         tmp[0:DHR, :])
                    nc.vector.tensor_add(kRt[1][DHR:P, nts], tmp2[DHR:P, :],
                                         tmp[DHR:P, :])

                emit_stats(0)
                emit_stats(1)
                load_weights()
                emit_proj(0)
                emit_stats(2)
                emit_proj(1)
                emit_stats(3)
                emit_proj(2)
                emit_proj(3)

            # ------------------------------------------------ phase C ----
            with (tc.tile_pool(name=f"pcm{sfx}", bufs=1) as pcm,
                  tc.tile_pool(name=f"pce{sfx}", bufs=4) as pex,
                  tc.tile_pool(name=f"pco{sfx}", bufs=3) as pot,
                  tc.tile_pool(name=f"pcps{sfx}", bufs=3, space="PSUM") as pcsc,
                  tc.tile_pool(name=f"pcpo{sfx}", bufs=3, space="PSUM") as pcso,
                  tc.tile_pool(name=f"pcpm{sfx}", bufs=2, space="PSUM") as pcss):
                maskT = pcm.tile([P, 4, NT], f32, name="maskT")
                nc.gpsimd.dma_start(maskT[:], mask_d[:])
                masks = [maskT[:, j, :] for j in range(4)]
                for h in range(4):
                    qRh = qRt[h // 2]
                    kRh = kRt[h % 2]
                    for qi in range(4):
                        qs = slice(qi * NT, (qi + 1) * NT)
                        nki = 4 * qi + 4
                        pso = pcso.tile([P, NT], f32, name="pso", tag="pso")
                        pss = pcss.tile([P, NT], f32, name="pss", tag="pss")
                        exs = [None] * nki

                        def emit_sc(ki):
                            ks = slice(ki * P, (ki + 1) * P)
                            psc = pcsc.tile([P, NT], f32, name="psc",
                                            tag="psc")
                            nc.tensor.matmul(psc[:], kT[h][:, ks],
                                             qT[h][:, qs],
                                             start=True, stop=False)
                            nc.tensor.matmul(psc[:], kRh[:, ks], qRh[:, qs],
                                             start=False, stop=True)
                            if ki >= 4 * qi:
                                nc.vector.tensor_add(psc[:], psc[:],
                                                     masks[ki - 4 * qi])
                            ex = pex.tile([P, NT], f16, name="ex", tag="ex")
                            nc.scalar.activation(ex[:], psc[:], AF.Exp,
                                                 scale=SCALE)
                            exs[ki] = ex

                        def emit_av(ki):
                            ex = exs[ki]
                            nc.tensor.matmul(pso[:],
                                             vS[:, ki, h * P:(h + 1) * P],
                                             ex[:], start=(ki == 0),
                                             stop=(ki == nki - 1))
                            nc.tensor.matmul(pss[:], ones_r, ex[:],
                                             start=(ki == 0),
                                             stop=(ki == nki - 1))

                        for j in range(nki + 3):
                            if j < nki:
                                emit_sc(j)
                            if j >= 3:
                                emit_av(j - 3)
                        rec = pot.tile([P, NT], f32, name="rec", tag="rec")
                        nc.vector.reciprocal(rec[:], pss[:])
                        ot = pot.tile([P, NT], f32, name="ot", tag="ot")
                        nc.vector.tensor_mul(ot[:], pso[:], rec[:])
                        otb = pot.tile([P, NT], f16, name="otb", tag="otb")
                        nc.vector.tensor_scalar_add(otb[:], ot[:],
                                                    bv[:, h:h + 1])
                        (nc.sync if qi % 2 == 0 else nc.scalar).dma_start(
                            o_sp[h][:, qs], otb[:])
                    if nocc:
                        nc.sync.dma_start(ogh[h][0], o_sp[h])
                    else:
                        nc.gpsimd.collective_compute(
                            "AllGather", mybir.AluOpType.bypass,
                            replica_groups=RG,
                            ins=[o_sp[h:h + 1].opt()], outs=[ogh[h][:].opt()])
                    if h == 1:
                        for p in range(2):
                            nc.sync.dma_start(
                                ogt0[:, :, p, :],
                                ogh[p][:, :, bass.ds(colo, NT)]
                                .transpose([1, 0, 2]))

            pprod_cm.__exit__(None, None, None)
            if dbg:
                with tc.tile_pool(name=f"pdbg{sfx}", bufs=4) as pdb:
                    for nm, tiles in [("d_qT", qT), ("d_kT", kT),
                                      ("d_qR", qRt), ("d_kR", kRt)]:
                        for i, tl in enumerate(tiles):
                            for half in range(2):
                                hs = slice(half * 1024, half * 1024 + 1024)
                                f = pdb.tile([P, 1024], f32, name="dmp",
                                             tag="dmp")
                                nc.vector.tensor_copy(f[:], tl[:, hs])
                                nc.sync.dma_start(dbg_d[nm][i, :, hs], f[:])
                    for i in range(16):
                        f = pdb.tile([P, 4 * P], f32, name="dmpv", tag="dmp")
                        nc.vector.tensor_copy(f[:], vS[:, i, :])
                        nc.sync.dma_start(dbg_d["d_vS"][:, i, :], f[:])
                    for hh in range(4):
                        for half in range(2):
                            hs = slice(half * 1024, half * 1024 + 1024)
                            f16t = pdb.tile([P, 1024], f16, name="dmp16",
                                            tag="dmp16")
                            nc.scalar.dma_start(f16t[:], o_sp[hh][:, hs])
                            f = pdb.tile([P, 1024], f32, name="dmpo",
                                         tag="dmp")
                            nc.vector.tensor_copy(f[:], f16t[:])
                            nc.sync.dma_start(dbg_d["d_o"][hh, :, hs], f[:])

            # ------------------------------------------------ phase D ----
            pmid_cm = tc.tile_pool(name=f"pmid{sfx}", bufs=1)
            pmid = pmid_cm.__enter__()
            xmid = [pmid.tile([P, NT], f32, name=f"xmid{mi}")
                    for mi in range(CC)]
            h2 = [pmid.tile([P, NT], f16, name=f"h2_{ci}")
                  for ci in range(CC)]
            pew_cm = tc.tile_pool(name=f"pew{sfx}", bufs=2)
            pew = pew_cm.__enter__()
            pdo_cm = tc.tile_pool(name=f"pdo{sfx}", bufs=1)
            pdo = pdo_cm.__enter__()
            with (tc.tile_pool(name=f"pdt{sfx}", bufs=2) as pdt,
                  tc.tile_pool(name=f"pdps{sfx}", bufs=4, space="PSUM") as pdps,
                  tc.tile_pool(name=f"pdst{sfx}", bufs=1, space="PSUM") as pdst):
                ogt1 = pdo.tile([P, 4, 2, NT], f16, name="ogt1")
                for p in range(2):
                    nc.sync.dma_start(
                        ogt1[:, :, p, :],
                        ogh[2 + p][:, :, bass.ds(colo, NT)]
                        .transpose([1, 0, 2]))
                xo = pdo.tile([P, CC, NT], f16, name="xo")
                woT = [woT01[:]]
                for mig in range(1, 4):
                    w = pdo.tile([P, CC, 4 * P], f16, name=f"woT{mig}")
                    nc.scalar.dma_start(w[:], wo_d[mig])
                    woT.append(w)
                nc.scalar.dma_start(xo[:], xown_d[:])
                for pas in range(2):
                    ogt = ogt0 if pas == 0 else ogt1
                    for mig in range(4):
                        wos = [woT[mig][:, 4 * kl + k2, :]
                               for kl in range(4) for k2 in range(4)]
                        for ml in range(4):
                            mi = mig * 4 + ml
                            ps = pdps.tile([P, NT], f32, name="pswo",
                                           tag="pswo")
                            for s in range(4):
                                for p in range(2):
                                    ki = 4 * s + 2 * pas + p
                                    nc.tensor.matmul(
                                        ps[:],
                                        wos[ki][:, ml * P:(ml + 1) * P],
                                        ogt[:, s, p, :],
                                        start=(s == 0 and p == 0),
                                        stop=(s == 3 and p == 1))
                            if pas == 0:
                                nc.vector.scalar_tensor_tensor(
                                    xmid[mi][:], ps[:], bo[:, mi:mi + 1],
                                    xo[:, mi, :], OP.add, OP.add)
                            else:
                                nc.vector.tensor_add(xmid[mi][:], xmid[mi][:],
                                                     ps[:])
                # LN2
                ps_mean = pdst.tile([P, NT], f32, name="psm2")
                ps_sq = pdst.tile([P, NT], f32, name="pss2")
                for ci in range(CC):
                    xm16 = pdt.tile([P, NT], f16, name="xm16", tag="xm16")
                    nc.scalar.activation(xm16[:], xmid[ci][:], AF.Identity)
                    sq = pdt.tile([P, NT], f16, name="sq2", tag="sq2")
                    nc.gpsimd.tensor_mul(sq[:], xm16[:], xm16[:])
                    nc.tensor.matmul(ps_mean[:], ones_r, xm16[:],
                                     start=(ci == 0), stop=(ci == CC - 1),
                                     skip_group_check=True)
                    nc.tensor.matmul(ps_sq[:], ones_r, sq[:],
                                     start=(ci == 0), stop=(ci == CC - 1),
                                     skip_group_check=True)
                meanb = pdt.tile([P, NT], f32, name="meanb2", bufs=1)
                nc.vector.tensor_scalar_mul(meanb[:], ps_mean[:], 1.0 / C)
                m2 = pdt.tile([P, NT], f32, name="m2_2", bufs=1)
                nc.vector.tensor_mul(m2[:], meanb[:], meanb[:])
                var = pdt.tile([P, NT], f32, name="var2", bufs=1)
                nc.vector.scalar_tensor_tensor(var[:], ps_sq[:], 1.0 / C,
                                               m2[:], OP.mult, OP.subtract)
                std = pdt.tile([P, NT], f32, name="std2", bufs=1)
                nc.scalar.activation(std[:], var[:], AF.Sqrt, bias=eps_t[:])
                rstd = pdt.tile([P, NT], f32, name="rstd2", bufs=1)
                nc.vector.reciprocal(rstd[:], std[:])
                for ci in range(CC):
                    eng = nc.gpsimd if ci % 2 else nc.vector
                    t1 = pdt.tile([P, NT], f32, name="t1b", tag="t1b")
                    eng.tensor_sub(t1[:], xmid[ci][:], meanb[:])
                    eng.tensor_mul(h2[ci][:], t1[:], rstd[:])
            pdo_cm.__exit__(None, None, None)

            if dbg:
                with tc.tile_pool(name=f"pdbg2{sfx}", bufs=4) as pdb:
                    for ci in range(CC):
                        nc.sync.dma_start(dbg_d["d_xmid"][ci], xmid[ci][:])
                        f = pdb.tile([P, NT], f32, name="dmp2", tag="dmp2")
                        nc.vector.tensor_copy(f[:], h2[ci][:])
                        nc.sync.dma_start(dbg_d["d_h2"][ci], f[:])

            # ------------------------------------------------ phase E ----
            with (tc.tile_pool(name=f"pew2{sfx}", bufs=3) as pew2,
                  tc.tile_pool(name=f"peg{sfx}", bufs=32) as peg,
                  tc.tile_pool(name=f"pea{sfx}", bufs=1) as pea,
                  tc.tile_pool(name=f"pet{sfx}", bufs=3) as pet,
                  tc.tile_pool(name=f"peps{sfx}", bufs=3, space="PSUM") as peps,
                  tc.tile_pool(name=f"pep2{sfx}", bufs=2, space="PSUM") as pep2):
                accs = [pea.tile([P, NT], f32, name=f"ffacc{mi}")
                        for mi in range(CC)]
                for hb in range(4):
                    gts = []
                    for mtg in range(4):
                        mtg_g = hb * 4 + mtg
                        wts = pew.tile([P, CC, 4 * P], f16, name="wf1",
                                       tag="wf1")
                        (nc.scalar if mtg % 2 else nc.gpsimd).dma_start(
                            wts[:], wff1_d[mtg_g])
                        for ml in range(4):
                            mt = mtg_g * 4 + ml
                            ps = peps.tile([P, NT], f32, name="psf1",
                                           tag="psf1")
                            for ci in range(CC):
                                nc.tensor.matmul(
                                    ps[:], wts[:, ci, ml * P:(ml + 1) * P],
                                    h2[ci][:], start=(ci == 0),
                                    stop=(ci == CC - 1))
                            gt = peg.tile([P, NT], f16, name="gt", tag="gt")
                            nc.scalar.activation(gt[:], ps[:],
                                                 AF.Gelu_apprx_tanh,
                                                 bias=bff1[:, mt:mt + 1])
                            gts.append(gt)
                    for mi in range(CC):
                        w2 = pew2.tile([P, CC * P], f16, name="wf2", tag="wf2")
                        (nc.scalar if mi % 2 else nc.gpsimd).dma_start(
                            w2[:], wff2_d[hb, mi])
                        ps2 = pep2.tile([P, NT], f32, name="psf2", tag="psf2")
                        for hl in range(CC):
                            nc.tensor.matmul(ps2[:],
                                             w2[:, hl * P:(hl + 1) * P],
                                             gts[hl][:], start=(hl == 0),
                                             stop=(hl == CC - 1))
                        if hb == 0:
                            nc.vector.tensor_copy(accs[mi][:], ps2[:])
                        elif hb < 3:
                            nc.vector.tensor_add(accs[mi][:], accs[mi][:],
                                                 ps2[:])
                        else:
                            acc2 = pet.tile([P, NT], f32, name="acc2",
                                            tag="acc2")
                            nc.vector.tensor_add(acc2[:], accs[mi][:], ps2[:])
                            ob = pet.tile([P, NT], f32, name="outb",
                                          tag="outb")
                            nc.vector.scalar_tensor_tensor(
                                ob[:], acc2[:], bff2[:, mi:mi + 1],
                                xmid[mi][:], OP.add, OP.add)
                            (nc.sync if mi % 2 else nc.scalar).dma_start(
                                outT_d[mi], ob[:])
            pew_cm.__exit__(None, None, None)
            pmid_cm.__exit__(None, None, None)
            pm0_cm.__exit__(None, None, None)

    nc.compile()
    return nc


# ------------------------------------------------------------------ host ---
def _rope_tables(r):
    """fp16 packed trig table [4, P, 6, NT] for core rank r."""
    t = np.arange(T, dtype=np.float64) + 1.0
    l = np.arange(DHR)
    cosq = np.zeros((2, P, T), np.float64)
    sinq = np.zeros((2, P, T), np.float64)
    for mt in range(2):
        for hl in range(2):
            h = 4 * r + 2 * mt + hl
            theta = 10000.0 ** (-2.0 * (32 * h + l // 2) / 1024.0)
            ang = t[None, :] * theta[:, None]            # [64, T]
            cosq[mt, 64 * hl:64 * hl + 64] = np.cos(ang)
            sinq[mt, 64 * hl:64 * hl + 64] = np.sin(ang)
    thk = 10000.0 ** (-2.0 * (l // 2) / 64.0)
    angk = t[None, :] * thk[:, None]
    cosk = np.concatenate([np.cos(angk)] * 2, axis=0)     # [128, T]
    sink = np.concatenate([np.sin(angk)] * 2, axis=0)
    trig = np.zeros((4, P, 6, NT), np.float16)
    for nt in range(4):
        ts_ = slice(nt * NT, (nt + 1) * NT)
        trig[nt, :, 0] = cosq[0, :, ts_]
        trig[nt, :, 1] = cosq[1, :, ts_]
        trig[nt, :, 2] = sinq[0, :, ts_]
        trig[nt, :, 3] = sinq[1, :, ts_]
        trig[nt, :, 4] = cosk[:, ts_]
        trig[nt, :, 5] = sink[:, ts_]
    return np.ascontiguousarray(trig)


def _shared_consts():
    r2 = np.zeros((P, P), np.float32)
    for i in range(64):
        r2[2 * i + 1, 2 * i] = -1.0
        r2[2 * i, 2 * i + 1] = 1.0
    mask = np.zeros((4, P, NT), np.float32)
    kl = np.arange(P)[:, None]
    ql = np.arange(NT)[None, :]
    for j in range(4):
        mask[j] = np.where(P * j + kl > ql, NEG, 0.0)
    ones = np.ones((P, P), np.float32)
    return r2, mask, ones


def prepare_in_maps(inputs):
    f32 = np.float32
    f16 = np.float16
    g = {k: np.asarray(v, f32) for k, v in inputs.items()}
    x = g["x"]
    r2, mask, ones = _shared_consts()
    mask_t = np.ascontiguousarray(mask.transpose(1, 0, 2))
    cf16 = np.ascontiguousarray(
        np.concatenate([ones, r2], axis=1).astype(f16))
    g1, be1 = g["ln1_scale"], g["ln1_bias"]
    g2, be2 = g["ln2_scale"], g["ln2_bias"]

    wd_kv, wd_q = g["w_down"][:, :LAT], g["w_down"][:, LAT:]
    bd_kv, bd_q = g["b_down"][:LAT], g["b_down"][LAT:]
    wuk, wuv = g["w_ukv"][:, :C], g["w_ukv"][:, C:]
    buk, buv = g["b_ukv"][:C], g["b_ukv"][C:]

    wo_t = np.ascontiguousarray(
        g["w_o"].reshape(CC, P, 4, 4 * P).transpose(2, 1, 0, 3).astype(f16))
    bo_t = g["b_o"].reshape(CC, P).T
    wff1 = g2[:, None] * g["w_ff1"]
    wff1_t = np.ascontiguousarray(
        wff1.reshape(CC, P, 16, 4 * P).transpose(2, 1, 0, 3).astype(f16))
    bff1_v = be2 @ g["w_ff1"] + g["b_ff1"]
    bff1_t = bff1_v.reshape(64, P).T
    wff2_t = np.ascontiguousarray(
        g["w_ff2"].reshape(4, CC, P, CC, P).transpose(0, 3, 2, 1, 4)
        .reshape(4, CC, P, CC * P).astype(f16))
    bff2_t = g["b_ff2"].reshape(CC, P).T

    in_maps = []
    for c in range(NCORES):
        b, r = divmod(c, 4)
        trig = _rope_tables(r)
        hs = slice(512 * r, 512 * (r + 1))           # head cols for this core
        wuq_s = g["w_uq"][:, hs]
        wuk_s = wuk[:, hs]
        wuv_s = wuv[:, hs]
        wq_e0 = wd_q @ wuq_s
        wk_e0 = wd_kv @ wuk_s
        wv_e0 = wd_kv @ wuv_s
        wq_e = g1[:, None] * wq_e0
        bq_e = be1 @ wq_e0 + bd_q @ wuq_s + g["b_uq"][hs]
        wk_e = g1[:, None] * wk_e0
        bk_e = be1 @ wk_e0 + bd_kv @ wuk_s + buk[hs]
        wv_e = g1[:, None] * wv_e0
        bv_e = be1 @ wv_e0 + bd_kv @ wuv_s + buv[hs]
        qrs = slice(256 * r, 256 * (r + 1))
        wqr_e = g1[:, None] * g["w_qr"][:, qrs]
        bqr_e = be1 @ g["w_qr"][:, qrs] + g["b_qr"][qrs]
        wkr_e = g1[:, None] * g["w_kr"]              # [C, 64]
        bkr_e = be1 @ g["w_kr"] + g["b_kr"]          # [64]
        wkr2 = np.zeros((2, C, P), f32)
        wkr2[0, :, :DHR] = wkr_e
        wkr2[0, :, DHR:] = wkr_e
        bkrD = np.concatenate([bkr_e, bkr_e])         # [P]

        cf32 = np.concatenate([
            bq_e.reshape(4, P).T, bk_e.reshape(4, P).T, bv_e.reshape(4, P).T,
            bqr_e.reshape(2, P).T, bkrD[:, None], np.zeros((P, 1), f32),
            bo_t, bff1_t, bff2_t], axis=1)
        assert cf32.shape == (P, 112)

        xs = x[b].T                                  # [C, T]
        m = {
            "xbf": np.ascontiguousarray(
                xs.reshape(CC, P, 4, NT).transpose(2, 1, 0, 3).astype(f16)),
            "xown16": np.ascontiguousarray(
                xs[:, 512 * r:512 * (r + 1)].reshape(CC, P, NT)
                .transpose(1, 0, 2).astype(f16)),
            "cf32": np.ascontiguousarray(cf32.astype(f32)),
            "cf16": cf16, "mask": mask_t,
            "wq": np.ascontiguousarray(
                wq_e.reshape(CC, P, 4 * P).transpose(1, 0, 2).astype(f16)),
            "wk": np.ascontiguousarray(
                wk_e.reshape(CC, P, 4 * P).transpose(1, 0, 2).astype(f16)),
            "wv": np.ascontiguousarray(
                wv_e.reshape(CC, P, 4 * P).transpose(1, 0, 2).astype(f16)),
            "wqr": np.ascontiguousarray(
                wqr_e.reshape(CC, P, 2 * P).transpose(1, 0, 2).astype(f16)),
            "wkr": np.ascontiguousarray(
                wkr2.reshape(2, CC, P, P).transpose(0, 2, 1, 3).astype(f16)),
            "trig": trig,
            "wo": wo_t,
            "wff1": wff1_t,
            "wff2": wff2_t,
        }
        in_maps.append(m)
    return in_maps


def assemble_output(results):
    out = np.zeros((B, T, C), np.float32)
    for c in range(NCORES):
        b, r = divmod(c, 4)
        o = results[c]["outT"].reshape(C, NT)
        out[b, NT * r:NT * (r + 1), :] = o.T
    return out


def kernel(**inputs):
    from concourse import bass_utils
    nc = _CACHE.get("nc")
    if nc is None:
        nc = build_program(repeat=1)
        _CACHE["nc"] = nc
    in_maps = prepare_in_maps(inputs)
    res = bass_utils.run_bass_kernel_spmd(nc, in_maps,
                                          core_ids=list(range(NCORES)))
    return assemble_output(res.results)


# revision 24
# speedup vs baseline: 9.8040x; 1.0949x over previous
"""Trainium2 Bass kernel for nn_Block_9268539425531 (MLA transformer block).

v4: 2 batch groups x 4-way head-TP within each group of 4 cores.
Per core (b = core//4, r = core%4, heads H = [4r, 4r+4)):
  Host folds w_down@w_ukv / w_down@w_uq (and the LN1/LN2 affine) into
  per-core effective fp16 weights, so each core computes LN1 for the full
  2048 tokens cheaply (matmul-trick stats, in-place normalize) and
  projects q/k/v/qR/kR for its own 4 heads directly -- no pre-attention
  collective.
  kR's 64-dim decoupled-RoPE uses two zero-padded weight variants
  (even/odd head parity) so score matmuls need no partition shifts.
  The v bias is applied post-softmax (softmax rows sum to 1).
  Attention runs head-outer; each head's output is AllGather'd (4 small
  fp16 collectives) as soon as it finishes, overlapped with the remaining
  heads' compute and with the first half of the w_o matmul (two
  accumulation passes over head pairs).
  FFN is token-sharded (512 tokens/core, full hidden dim).
All matmul operands fp16 (full rate on TRN2), accumulation fp32.
All bulk DRAM tensors are host-pre-transposed to partition-major layouts
so every DMA moves long contiguous runs; scalar constants ride in two
packed tensors (one f32, one f16).
"""
import math
import numpy as np

B, T, C = 2, 2048, 2048
NH = 16
DK = 128
DHR = 64
LAT = 512
P = 128
NT = 512           # tokens per core
CC = C // P        # 16
NCORES = 8
SCALE = 1.0 / math.sqrt(DK)
NEG = -1.0e9
RG = [[0, 1, 2, 3], [4, 5, 6, 7]]

_CACHE = {}


# ---------------------------------------------------------------- program ---
def build_program(repeat=1, nocc=False, dbg=False):
    from contextlib import ExitStack
    from concourse import bass, bacc, tile, mybir

    dt = mybir.dt
    f32 = dt.float32
    f16 = dt.float16
    AF = mybir.ActivationFunctionType
    OP = mybir.AluOpType

    nc = bacc.Bacc("TRN2", target_bir_lowering=False, debug=False,
                   num_devices=NCORES)

    def din(name, shape, dtype=f16):
        return nc.dram_tensor(name, shape, dtype, kind="ExternalInput")

    xbf_d = din("xbf", [4, P, CC, NT])                # full x by token block
    xown_d = din("xown16", [P, CC, NT])               # own block, fp16
    cf32_d = din("cf32", [P, 112], f32)               # packed f32 consts
    cf16_d = din("cf16", [P, 2 * P])                  # ones | r2
    mask_d = din("mask", [P, 4, NT], f32)
    wq_d = din("wq", [P, CC, 4 * P])
    wk_d = din("wk", [P, CC, 4 * P])
    wv_d = din("wv", [P, CC, 4 * P])
    wqr_d = din("wqr", [P, CC, 2 * P])
    wkr_d = din("wkr", [2, P, CC, P])                 # even/odd zero-padded
    trig_d = din("trig", [4, P, 6, NT])               # cq0 cq1 sq0 sq1 ck sk
    wo_d = din("wo", [4, P, CC, 4 * P])               # [mig, P, ki, 512]
    wff1_d = din("wff1", [16, P, CC, 4 * P])          # [mtg, P, ci, 512]
    wff2_d = din("wff2", [4, CC, P, CC * P])          # [hb, mi, P, 2048]
    outT_d = nc.dram_tensor("outT", [CC, P, NT], f32, kind="ExternalOutput")

    dbg_d = {}
    if dbg:
        for nm, shape in [("d_qT", [4, P, T]), ("d_kT", [4, P, T]),
                          ("d_vS", [P, 16, 4 * P]), ("d_qR", [2, P, T]),
                          ("d_kR", [2, P, T]), ("d_o", [4, P, T]),
                          ("d_xmid", [CC, P, NT]), ("d_h2", [CC, P, NT]),
                          ("d_h1", [CC, P, T])]:
            dbg_d[nm] = nc.dram_tensor(nm, shape, f32, kind="ExternalOutput")

    with tile.TileContext(nc) as tc, ExitStack() as ctx:
        pc = ctx.enter_context(tc.tile_pool(name="const", bufs=1))
        pdram = ctx.enter_context(tc.tile_pool(name="dram", bufs=1, space="DRAM"))

        cb = pc.tile([P, 112], f32, name="cb")
        nc.sync.dma_start(cb[:], cf32_d[:])
        c16 = pc.tile([P, 2 * P], f16, name="c16")
        nc.sync.dma_start(c16[:], cf16_d[:])
        bq = cb[:, 0:4]
        bk = cb[:, 4:8]
        bv = cb[:, 8:12]
        bqr = cb[:, 12:14]
        bkrD = cb[:, 14:15]
        bo = cb[:, 16:32]
        bff1 = cb[:, 32:96]
        bff2 = cb[:, 96:112]
        ones_r = c16[:, 0:P]
        r2 = c16[:, P:2 * P]
        eps_t = pc.tile([P, 1], f32, name="eps_t")
        nc.vector.memset(eps_t[:], 1e-6)

        o_sp = pdram.tile([4, P, T], f16, name="o_sp")
        ogh = [pdram.tile([4, P, T], f16, name=f"ogh{h}") for h in range(4)]

        pid = nc.sync.partition_id()
        colo = (pid % 4) * NT

        for rep in range(repeat):
            sfx = f"r{rep}"
            # ogt0 staged early so the first AllGather's output can stream
            # into SBUF while heads 2-3 still compute
            pm0_cm = tc.tile_pool(name=f"pm0{sfx}", bufs=1)
            pm0 = pm0_cm.__enter__()
            ogt0 = pm0.tile([P, 4, 2, NT], f16, name="ogt0")
            woT01 = pm0.tile([P, CC, 4 * P], f16, name="woT01")
            # persistent across AB + C
            pprod_cm = tc.tile_pool(name=f"prod{sfx}", bufs=1)
            pprod = pprod_cm.__enter__()
            qT = [pprod.tile([P, T], f16, name=f"qT{m}") for m in range(4)]
            kT = [pprod.tile([P, T], f16, name=f"kT{m}") for m in range(4)]
            vS = pprod.tile([P, 16, 4 * P], f16, name="vS")
            qRt = [pprod.tile([P, T], f16, name=f"qR{m}") for m in range(2)]
            kRt = [pprod.tile([P, T], f16, name=f"kR{m}") for m in range(2)]
            nc.vector.memset(kRt[0][DHR:P, :], 0.0)
            nc.vector.memset(kRt[1][0:DHR, :], 0.0)

            # ------------------------------------------------ phase AB ----
            with (tc.tile_pool(name=f"pabw{sfx}", bufs=1) as pw,
                  tc.tile_pool(name=f"pabx{sfx}", bufs=2) as px,
                  tc.tile_pool(name=f"pabsq{sfx}", bufs=2) as psq,
                  tc.tile_pool(name=f"pabcs{sfx}", bufs=1) as pcs,
                  tc.tile_pool(name=f"pabt{sfx}", bufs=2) as pt,
                  tc.tile_pool(name=f"pabps{sfx}", bufs=2, space="PSUM") as pps,
                  tc.tile_pool(name=f"pabpp{sfx}", bufs=2, space="PSUM") as ppp):
                wq_sb = wk_sb = wv_sb = wqr_sb = wkrD_sb = None

                def load_weights():
                    nonlocal wq_sb, wk_sb, wv_sb, wqr_sb, wkrD_sb
                    wqT = pw.tile([P, CC, 4 * P], f16, name="wqT")
                    nc.sync.dma_start(wqT[:], wq_d[:])
                    wkT = pw.tile([P, CC, 4 * P], f16, name="wkT")
                    nc.sync.dma_start(wkT[:], wk_d[:])
                    wvT = pw.tile([P, CC, 4 * P], f16, name="wvT")
                    nc.sync.dma_start(wvT[:], wv_d[:])
                    wrT = pw.tile([P, CC, 2 * P], f16, name="wrT")
                    nc.sync.dma_start(wrT[:], wqr_d[:])
                    wkdT = pw.tile([P, CC, P], f16, name="wkdT")
                    nc.sync.dma_start(wkdT[:], wkr_d[0])
                    wq_sb = [wqT[:, ci, :] for ci in range(CC)]
                    wk_sb = [wkT[:, ci, :] for ci in range(CC)]
                    wv_sb = [wvT[:, ci, :] for ci in range(CC)]
                    wqr_sb = [wrT[:, ci, :] for ci in range(CC)]
                    wkrD_sb = [wkdT[:, ci, :] for ci in range(CC)]
                    nc.sync.dma_start(woT01[:], wo_d[0])

                def rope(pre, cos_t, sin_t, dst_ap):
                    rot = ppp.tile([P, NT], f32, name="psrot", tag="psrot",
                                   bufs=1)
                    nc.tensor.matmul(rot[:], r2, pre[:], start=True,
                                     stop=True)
                    tmp = pt.tile([P, NT], f16, name="rtmp", tag="rtmp",
                                  bufs=1)
                    nc.vector.tensor_mul(tmp[:], rot[:], sin_t)
                    tmp2 = pt.tile([P, NT], f16, name="rtmp2", tag="rtmp2",
                                   bufs=1)
                    nc.vector.tensor_mul(tmp2[:], pre[:], cos_t)
                    nc.vector.tensor_add(dst_ap, tmp2[:], tmp[:])

                hts = [None] * 4   # per-nt normalized-x tiles (in-place)

                def emit_stats(nt):
                    xb = px.tile([P, CC, NT], f16, name="xb", tag="xb")
                    nc.sync.dma_start(xb[:, 0:8, :], xbf_d[nt, :, 0:8, :])
                    nc.sync.dma_start(xb[:, 8:CC, :], xbf_d[nt, :, 8:CC, :])
                    ps_mean = pps.tile([P, NT], f32, name="psm", tag="psm")
                    ps_sq = pps.tile([P, NT], f32, name="pss", tag="pss")
                    for ci in range(CC):
                        sq = psq.tile([P, NT], f16, name="sq", tag="sq")
                        if ci % 2 == 0:
                            nc.scalar.square(sq[:], xb[:, ci, :])
                        else:
                            nc.gpsimd.tensor_mul(sq[:], xb[:, ci, :],
                                                 xb[:, ci, :])
                        nc.tensor.matmul(ps_mean[:], ones_r, xb[:, ci, :],
                                         start=(ci == 0), stop=(ci == CC - 1),
                                         skip_group_check=True)
                        nc.tensor.matmul(ps_sq[:], ones_r, sq[:],
                                         start=(ci == 0), stop=(ci == CC - 1),
                                         skip_group_check=True)
                    meanb = pt.tile([P, NT], f32, name="meanb", tag="meanb")
                    nc.vector.tensor_scalar_mul(meanb[:], ps_mean[:], 1.0 / C)
                    m2 = pt.tile([P, NT], f32, name="m2", tag="m2", bufs=1)
                    nc.vector.tensor_mul(m2[:], meanb[:], meanb[:])
                    var = pt.tile([P, NT], f32, name="var", tag="var",
                                  bufs=1)
                    nc.vector.scalar_tensor_tensor(var[:], ps_sq[:], 1.0 / C,
                                                   m2[:], OP.mult, OP.subtract)
                    std = pt.tile([P, NT], f32, name="std", tag="m2",
                                  bufs=1)
                    nc.scalar.activation(std[:], var[:], AF.Sqrt,
                                         bias=eps_t[:])
                    rstd = pt.tile([P, NT], f32, name="rstd", tag="rstd")
                    nc.vector.reciprocal(rstd[:], std[:])
                    # normalize in place: xb <- (xb - mean) * rstd
                    for ci in range(CC):
                        eng = nc.gpsimd if ci % 4 == 3 else nc.vector
                        t1 = psq.tile([P, NT], f16, name="t1", tag="t1")
                        eng.tensor_sub(t1[:], xb[:, ci, :], meanb[:])
                        eng.tensor_mul(xb[:, ci, :], t1[:], rstd[:])
                    hts[nt] = xb

                def emit_proj(nt):
                    nts = slice(nt * NT, (nt + 1) * NT)
                    hb = hts[nt]
                    h = [hb[:, ci, :] for ci in range(CC)]
                    trig = pcs.tile([P, 6, NT], f16, name="trig", tag="trig")
                    nc.scalar.dma_start(trig[:], trig_d[nt])
                    if dbg:
                        for ci in range(CC):
                            hf = pt.tile([P, NT], f32, name="hdump",
                                         tag="hdump", bufs=2)
                            nc.vector.tensor_copy(hf[:], h[ci])
                            nc.sync.dma_start(dbg_d["d_h1"][ci, :, nts], hf[:])
                    for m in range(4):
                        ps = ppp.tile([P, NT], f32, name="psp", tag="psp")
                        for ci in range(CC):
                            nc.tensor.matmul(ps[:],
                                             wq_sb[ci][:, m * P:(m + 1) * P],
                                             h[ci], start=(ci == 0),
                                             stop=(ci == CC - 1))
                        nc.scalar.activation(qT[m][:, nts], ps[:], AF.Identity,
                                             bias=bq[:, m:m + 1])
                    for m in range(4):
                        ps = ppp.tile([P, NT], f32, name="psp", tag="psp")
                        for ci in range(CC):
                            nc.tensor.matmul(ps[:],
                                             wk_sb[ci][:, m * P:(m + 1) * P],
                                             h[ci], start=(ci == 0),
                                             stop=(ci == CC - 1))
                        nc.scalar.activation(kT[m][:, nts], ps[:], AF.Identity,
                                             bias=bk[:, m:m + 1])
                    for tt in range(4):
                        ps = ppp.tile([P, 4 * P], f32, name="psp", tag="psp")
                        for ci in range(CC):
                            nc.tensor.matmul(ps[:],
                                             h[ci][:, tt * P:(tt + 1) * P],
                                             wv_sb[ci], start=(ci == 0),
                                             stop=(ci == CC - 1))
                        nc.scalar.activation(vS[:, 4 * nt + tt, :], ps[:],
                                             AF.Identity)
                    # qR (2 chunks = 4 heads, 64-dim pairs on partitions)
                    for mt in range(2):
                        ps = ppp.tile([P, NT], f32, name="psp", tag="psp")
                        for ci in range(CC):
                            nc.tensor.matmul(ps[:],
                                             wqr_sb[ci][:, mt * P:(mt + 1) * P],
                                             h[ci], start=(ci == 0),
                                             stop=(ci == CC - 1))
                        pre = pt.tile([P, NT], f16, name="pre", tag="pre",
                                      bufs=1)
                        nc.scalar.activation(pre[:], ps[:], AF.Identity,
                                             bias=bqr[:, mt:mt + 1])
                        rope(pre, trig[:, mt, :], trig[:, 2 + mt, :],
                             qRt[mt][:, nts])
                    # kR: one duplicated-layout chain; the per-parity
                    # zero halves are static (memset once per rep)
                    ps = ppp.tile([P, NT], f32, name="psp", tag="psp")
                    for ci in range(CC):
                        nc.tensor.matmul(ps[:], wkrD_sb[ci], h[ci],
                                         start=(ci == 0),
                                         stop=(ci == CC - 1))
                    pre = pt.tile([P, NT], f16, name="pre", tag="pre",
                                  bufs=1)
                    nc.scalar.activation(pre[:], ps[:], AF.Identity,
                                         bias=bkrD)
                    rot = ppp.tile([P, NT], f32, name="psrot", tag="psrot",
                                   bufs=1)
                    nc.tensor.matmul(rot[:], r2, pre[:], start=True, stop=True)
                    tmp = pt.tile([P, NT], f16, name="rtmp", tag="rtmp",
                                  bufs=1)
                    nc.vector.tensor_mul(tmp[:], rot[:], trig[:, 5, :])
                    tmp2 = pt.tile([P, NT], f16, name="rtmp2", tag="rtmp2",
                                   bufs=1)
                    nc.vector.tensor_mul(tmp2[:], pre[:], trig[:, 4, :])
                    nc.vector.tensor_add(kRt[0][0:DHR, nts], tmp2[0:DHR, :],
                                         tmp[0:DHR, :])
                    nc.vector.tensor_add(kRt[1][DHR:P, nts], tmp2[DHR:P, :],
                                         tmp[DHR:P, :])

                emit_stats(0)
                emit_stats(1)
                load_weights()
                emit_proj(0)
                emit_stats(2)
                emit_proj(1)
                emit_stats(3)
                emit_proj(2)
                emit_proj(3)

            # ------------------------------------------------ phase C ----
            with (tc.tile_pool(name=f"pcm{sfx}", bufs=1) as pcm,
                  tc.tile_pool(name=f"pce{sfx}", bufs=4) as pex,
                  tc.tile_pool(name=f"pco{sfx}", bufs=3) as pot,
                  tc.tile_pool(name=f"pcps{sfx}", bufs=3, space="PSUM") as pcsc,
                  tc.tile_pool(name=f"pcpo{sfx}", bufs=3, space="PSUM") as pcso,
                  tc.tile_pool(name=f"pcpm{sfx}", bufs=2, space="PSUM") as pcss):
                maskT = pcm.tile([P, 4, NT], f32, name="maskT")
                nc.gpsimd.dma_start(maskT[:], mask_d[:])
                masks = [maskT[:, j, :] for j in range(4)]
                for h in range(4):
                    qRh = qRt[h // 2]
                    kRh = kRt[h % 2]
                    for qi in range(4):
                        qs = slice(qi * NT, (qi + 1) * NT)
                        nki = 4 * qi + 4
                        pso = pcso.tile([P, NT], f32, name="pso", tag="pso")
                        pss = pcss.tile([P, NT], f32, name="pss", tag="pss")
                        exs = [None] * nki

                        def emit_sc(ki):
                            ks = slice(ki * P, (ki + 1) * P)
                            psc = pcsc.tile([P, NT], f32, name="psc",
                                            tag="psc")
                            nc.tensor.matmul(psc[:], kT[h][:, ks],
                                             qT[h][:, qs],
                                             start=True, stop=False)
                            nc.tensor.matmul(psc[:], kRh[:, ks], qRh[:, qs],
                                             start=False, stop=True)
                            if ki >= 4 * qi:
                                nc.vector.tensor_add(psc[:], psc[:],
                                                     masks[ki - 4 * qi])
                            ex = pex.tile([P, NT], f16, name="ex", tag="ex")
                            nc.scalar.activation(ex[:], psc[:], AF.Exp,
                                                 scale=SCALE)
                            exs[ki] = ex

                        def emit_av(ki):
                            ex = exs[ki]
                            nc.tensor.matmul(pso[:],
                                             vS[:, ki, h * P:(h + 1) * P],
                                             ex[:], start=(ki == 0),
                                             stop=(ki == nki - 1))
                            nc.tensor.matmul(pss[:], ones_r, ex[:],
                                             start=(ki == 0),
                                             stop=(ki == nki - 1))

                        for j in range(nki + 3):
                            if j < nki:
                                emit_sc(j)
                            if j >= 3:
                                emit_av(j - 3)
                        rec = pot.tile([P, NT], f32, name="rec", tag="rec")
                        nc.vector.reciprocal(rec[:], pss[:])
                        ot = pot.tile([P, NT], f32, name="ot", tag="ot")
                        nc.vector.tensor_mul(ot[:], pso[:], rec[:])
                        otb = pot.tile([P, NT], f16, name="otb", tag="otb")
                        nc.vector.tensor_scalar_add(otb[:], ot[:],
                                                    bv[:, h:h + 1])
                        (nc.sync if qi % 2 == 0 else nc.scalar).dma_start(
                            o_sp[h][:, qs], otb[:])
                    if nocc:
                        nc.sync.dma_start(ogh[h][0], o_sp[h])
                    else:
                        nc.gpsimd.collective_compute(
                            "AllGather", mybir.AluOpType.bypass,
                            replica_groups=RG,
                            ins=[o_sp[h:h + 1].opt()], outs=[ogh[h][:].opt()])
                    if h == 1:
                        for p in range(2):
                            nc.sync.dma_start(
                                ogt0[:, :, p, :],
                                ogh[p][:, :, bass.ds(colo, NT)]
                                .transpose([1, 0, 2]))

            pprod_cm.__exit__(None, None, None)
            if dbg:
                with tc.tile_pool(name=f"pdbg{sfx}", bufs=4) as pdb:
                    for nm, tiles in [("d_qT", qT), ("d_kT", kT),
                                      ("d_qR", qRt), ("d_kR", kRt)]:
                        for i, tl in enumerate(tiles):
                            for half in range(2):
                                hs = slice(half * 1024, half * 1024 + 1024)
                                f = pdb.tile([P, 1024], f32, name="dmp",
                                             tag="dmp")
                                nc.vector.tensor_copy(f[:], tl[:, hs])
                                nc.sync.dma_start(dbg_d[nm][i, :, hs], f[:])
                    for i in range(16):
                        f = pdb.tile([P, 4 * P], f32, name="dmpv", tag="dmp")
                        nc.vector.tensor_copy(f[:], vS[:, i, :])
                        nc.sync.dma_start(dbg_d["d_vS"][:, i, :], f[:])
                    for hh in range(4):
                        for half in range(2):
                            hs = slice(half * 1024, half * 1024 + 1024)
                            f16t = pdb.tile([P, 1024], f16, name="dmp16",
                                            tag="dmp16")
                            nc.scalar.dma_start(f16t[:], o_sp[hh][:, hs])
                            f = pdb.tile([P, 1024], f32, name="dmpo",
                                         tag="dmp")
                            nc.vector.tensor_copy(f[:], f16t[:])
                            nc.sync.dma_start(dbg_d["d_o"][hh, :, hs], f[:])

            # ------------------------------------------------ phase D ----
            pmid_cm = tc.tile_pool(name=f"pmid{sfx}", bufs=1)
            pmid = pmid_cm.__enter__()
            xmid = [pmid.tile([P, NT], f32, name=f"xmid{mi}")
                    for mi in range(CC)]
            h2 = [pmid.tile([P, NT], f16, name=f"h2_{ci}")
                  for ci in range(CC)]
            pew_cm = tc.tile_pool(name=f"pew{sfx}", bufs=2)
            pew = pew_cm.__enter__()
            pdo_cm = tc.tile_pool(name=f"pdo{sfx}", bufs=1)
            pdo = pdo_cm.__enter__()
            with (tc.tile_pool(name=f"pdt{sfx}", bufs=2) as pdt,
                  tc.tile_pool(name=f"pdps{sfx}", bufs=4, space="PSUM") as pdps,
                  tc.tile_pool(name=f"pdst{sfx}", bufs=1, space="PSUM") as pdst):
                ogt1 = pdo.tile([P, 4, 2, NT], f16, name="ogt1")
                for p in range(2):
                    nc.sync.dma_start(
                        ogt1[:, :, p, :],
                        ogh[2 + p][:, :, bass.ds(colo, NT)]
                        .transpose([1, 0, 2]))
                xo = pdo.tile([P, CC, NT], f16, name="xo")
                woT = [woT01[:]]
                for mig in range(1, 4):
                    w = pdo.tile([P, CC, 4 * P], f16, name=f"woT{mig}")
                    nc.scalar.dma_start(w[:], wo_d[mig])
                    woT.append(w)
                nc.scalar.dma_start(xo[:], xown_d[:])
                for pas in range(2):
                    ogt = ogt0 if pas == 0 else ogt1
                    for mig in range(4):
                        wos = [woT[mig][:, 4 * kl + k2, :]
                               for kl in range(4) for k2 in range(4)]
                        for ml in range(4):
                            mi = mig * 4 + ml
                            ps = pdps.tile([P, NT], f32, name="pswo",
                                           tag="pswo")
                            for s in range(4):
                                for p in range(2):
                                    ki = 4 * s + 2 * pas + p
                                    nc.tensor.matmul(
                                        ps[:],
                                        wos[ki][:, ml * P:(ml + 1) * P],
                                        ogt[:, s, p, :],
                                        start=(s == 0 and p == 0),
                                        stop=(s == 3 and p == 1))
                            if pas == 0:
                                nc.vector.scalar_tensor_tensor(
                                    xmid[mi][:], ps[:], bo[:, mi:mi + 1],
                                    xo[:, mi, :], OP.add, OP.add)
                            else:
                                nc.vector.tensor_add(xmid[mi][:], xmid[mi][:],
                                                     ps[:])
                # LN2
                ps_mean = pdst.tile([P, NT], f32, name="psm2")
                ps_sq = pdst.tile([P, NT], f32, name="pss2")
                for ci in range(CC):
                    xm16 = pdt.tile([P, NT], f16, name="xm16", tag="xm16")
                    nc.scalar.activation(xm16[:], xmid[ci][:], AF.Identity)
                    sq = pdt.tile([P, NT], f16, name="sq2", tag="sq2")
                    nc.gpsimd.tensor_mul(sq[:], xm16[:], xm16[:])
                    nc.tensor.matmul(ps_mean[:], ones_r, xm16[:],
                                     start=(ci == 0), stop=(ci == CC - 1),
                                     skip_group_check=True)
                    nc.tensor.matmul(ps_sq[:], ones_r, sq[:],
                                     start=(ci == 0), stop=(ci == CC - 1),
                                     skip_group_check=True)
                meanb = pdt.tile([P, NT], f32, name="meanb2", bufs=1)
                nc.vector.tensor_scalar_mul(meanb[:], ps_mean[:], 1.0 / C)
                m2 = pdt.tile([P, NT], f32, name="m2_2", bufs=1)
                nc.vector.tensor_mul(m2[:], meanb[:], meanb[:])
                var = pdt.tile([P, NT], f32, name="var2", bufs=1)
                nc.vector.scalar_tensor_tensor(var[:], ps_sq[:], 1.0 / C,
                                               m2[:], OP.mult, OP.subtract)
                std = pdt.tile([P, NT], f32, name="std2", bufs=1)
                nc.scalar.activation(std[:], var[:], AF.Sqrt, bias=eps_t[:])
                rstd = pdt.tile([P, NT], f32, name="rstd2", bufs=1)
                nc.vector.reciprocal(rstd[:], std[:])
                for ci in range(CC):
                    eng = nc.gpsimd if ci % 2 else nc.vector
                    t1 = pdt.tile([P, NT], f32, name="t1b", tag="t1b")
                    eng.tensor_sub(t1[:], xmid[ci][:], meanb[:])
                    eng.tensor_mul(h2[ci][:], t1[:], rstd[:])
            pdo_cm.__exit__(None, None, None)

            if dbg:
                with tc.tile_pool(name=f"pdbg2{sfx}", bufs=4) as pdb:
                    for ci in range(CC):
                        nc.sync.dma_start(dbg_d["d_xmid"][ci], xmid[ci][:])
                        f = pdb.tile([P, NT], f32, name="dmp2", tag="dmp2")
                        nc.vector.tensor_copy(f[:], h2[ci][:])
                        nc.sync.dma_start(dbg_d["d_h2"][ci], f[:])

            # ------------------------------------------------ phase E ----
            with (tc.tile_pool(name=f"pew2{sfx}", bufs=3) as pew2,
                  tc.tile_pool(name=f"peg{sfx}", bufs=32) as peg,
                  tc.tile_pool(name=f"pea{sfx}", bufs=1) as pea,
                  tc.tile_pool(name=f"pet{sfx}", bufs=3) as pet,
                  tc.tile_pool(name=f"peps{sfx}", bufs=3, space="PSUM") as peps,
                  tc.tile_pool(name=f"pep2{sfx}", bufs=2, space="PSUM") as pep2):
                accs = [pea.tile([P, NT], f32, name=f"ffacc{mi}")
                        for mi in range(CC)]
                for hb in range(4):
                    gts = []
                    for mtg in range(4):
                        mtg_g = hb * 4 + mtg
                        wts = pew.tile([P, CC, 4 * P], f16, name="wf1",
                                       tag="wf1")
                        (nc.scalar if mtg % 2 else nc.gpsimd).dma_start(
                            wts[:], wff1_d[mtg_g])
                        for ml in range(4):
                            mt = mtg_g * 4 + ml
                            ps = peps.tile([P, NT], f32, name="psf1",
                                           tag="psf1")
                            for ci in range(CC):
                                nc.tensor.matmul(
                                    ps[:], wts[:, ci, ml * P:(ml + 1) * P],
                                    h2[ci][:], start=(ci == 0),
                                    stop=(ci == CC - 1))
                            gt = peg.tile([P, NT], f16, name="gt", tag="gt")
                            nc.scalar.activation(gt[:], ps[:],
                                                 AF.Gelu_apprx_tanh,
                                                 bias=bff1[:, mt:mt + 1])
                            gts.append(gt)
                    for mi in range(CC):
                        w2 = pew2.tile([P, CC * P], f16, name="wf2", tag="wf2")
                        (nc.scalar if mi % 2 else nc.gpsimd).dma_start(
                            w2[:], wff2_d[hb, mi])
                        ps2 = pep2.tile([P, NT], f32, name="psf2", tag="psf2")
                        for hl in range(CC):
                            nc.tensor.matmul(ps2[:],
                                             w2[:, hl * P:(hl + 1) * P],
                                             gts[hl][:], start=(hl == 0),
                                             stop=(hl == CC - 1))
                        if hb == 0:
                            nc.vector.tensor_copy(accs[mi][:], ps2[:])
                        elif hb < 3:
                            nc.vector.tensor_add(accs[mi][:], accs[mi][:],
                                                 ps2[:])
                        else:
                            acc2 = pet.tile([P, NT], f32, name="acc2",
                                            tag="acc2")
                            nc.vector.tensor_add(acc2[:], accs[mi][:], ps2[:])
                            ob = pet.tile([P, NT], f32, name="outb",
                                          tag="outb")
                            nc.vector.scalar_tensor_tensor(
                                ob[:], acc2[:], bff2[:, mi:mi + 1],
                                xmid[mi][:], OP.add, OP.add)
                            (nc.sync if mi % 2 else nc.scalar).dma_start(
                                outT_d[mi], ob[:])
            pew_cm.__exit__(None, None, None)
            pmid_cm.__exit__(None, None, None)
            pm0_cm.__exit__(None, None, None)

    nc.compile()
    return nc


# ------------------------------------------------------------------ host ---
def _rope_tables(r):
    """fp16 packed trig table [4, P, 6, NT] for core rank r."""
    t = np.arange(T, dtype=np.float64) + 1.0
    l = np.arange(DHR)
    cosq = np.zeros((2, P, T), np.float64)
    sinq = np.zeros((2, P, T), np.float64)
    for mt in range(2):
        for hl in range(2):
            h = 4 * r + 2 * mt + hl
            theta = 10000.0 ** (-2.0 * (32 * h + l // 2) / 1024.0)
            ang = t[None, :] * theta[:, None]            # [64, T]
            cosq[mt, 64 * hl:64 * hl + 64] = np.cos(ang)
            sinq[mt, 64 * hl:64 * hl + 64] = np.sin(ang)
    thk = 10000.0 ** (-2.0 * (l // 2) / 64.0)
    angk = t[None, :] * thk[:, None]
    cosk = np.concatenate([np.cos(angk)] * 2, axis=0)     # [128, T]
    sink = np.concatenate([np.sin(angk)] * 2, axis=0)
    trig = np.zeros((4, P, 6, NT), np.float16)
    for nt in range(4):
        ts_ = slice(nt * NT, (nt + 1) * NT)
        trig[nt, :, 0] = cosq[0, :, ts_]
        trig[nt, :, 1] = cosq[1, :, ts_]
        trig[nt, :, 2] = sinq[0, :, ts_]
        trig[nt, :, 3] = sinq[1, :, ts_]
        trig[nt, :, 4] = cosk[:, ts_]
        trig[nt, :, 5] = sink[:, ts_]
    return np.ascontiguousarray(trig)


def _shared_consts():
    r2 = np.zeros((P, P), np.float32)
    for i in range(64):
        r2[2 * i + 1, 2 * i] = -1.0
        r2[2 * i, 2 * i + 1] = 1.0
    mask = np.zeros((4, P, NT), np.float32)
    kl = np.arange(P)[:, None]
    ql = np.arange(NT)[None, :]
    for j in range(4):
        mask[j] = np.where(P * j + kl > ql, NEG, 0.0)
    ones = np.ones((P, P), np.float32)
    return r2, mask, ones


def prepare_in_maps(inputs):
    f32 = np.float32
    f16 = np.float16
    g = {k: np.asarray(v, f32) for k, v in inputs.items()}
    x = g["x"]
    r2, mask, ones = _shared_consts()
    mask_t = np.ascontiguousarray(mask.transpose(1, 0, 2))
    cf16 = np.ascontiguousarray(
        np.concatenate([ones, r2], axis=1).astype(f16))
    g1, be1 = g["ln1_scale"], g["ln1_bias"]
    g2, be2 = g["ln2_scale"], g["ln2_bias"]

    wd_kv, wd_q = g["w_down"][:, :LAT], g["w_down"][:, LAT:]
    bd_kv, bd_q = g["b_down"][:LAT], g["b_down"][LAT:]
    wuk, wuv = g["w_ukv"][:, :C], g["w_ukv"][:, C:]
    buk, buv = g["b_ukv"][:C], g["b_ukv"][C:]

    wo_t = np.ascontiguousarray(
        g["w_o"].reshape(CC, P, 4, 4 * P).transpose(2, 1, 0, 3).astype(f16))
    bo_t = g["b_o"].reshape(CC, P).T
    wff1 = g2[:, None] * g["w_ff1"]
    wff1_t = np.ascontiguousarray(
        wff1.reshape(CC, P, 16, 4 * P).transpose(2, 1, 0, 3).astype(f16))
    bff1_v = be2 @ g["w_ff1"] + g["b_ff1"]
    bff1_t = bff1_v.reshape(64, P).T
    wff2_t = np.ascontiguousarray(
        g["w_ff2"].reshape(4, CC, P, CC, P).transpose(0, 3, 2, 1, 4)
        .reshape(4, CC, P, CC * P).astype(f16))
    bff2_t = g["b_ff2"].reshape(CC, P).T

    in_maps = []
    for c in range(NCORES):
        b, r = divmod(c, 4)
        trig = _rope_tables(r)
        hs = slice(512 * r, 512 * (r + 1))           # head cols for this core
        wuq_s = g["w_uq"][:, hs]
        wuk_s = wuk[:, hs]
        wuv_s = wuv[:, hs]
        wq_e0 = wd_q @ wuq_s
        wk_e0 = wd_kv @ wuk_s
        wv_e0 = wd_kv @ wuv_s
        wq_e = g1[:, None] * wq_e0
        bq_e = be1 @ wq_e0 + bd_q @ wuq_s + g["b_uq"][hs]
        wk_e = g1[:, None] * wk_e0
        bk_e = be1 @ wk_e0 + bd_kv @ wuk_s + buk[hs]
        wv_e = g1[:, None] * wv_e0
        bv_e = be1 @ wv_e0 + bd_kv @ wuv_s + buv[hs]
        qrs = slice(256 * r, 256 * (r + 1))
        wqr_e = g1[:, None] * g["w_qr"][:, qrs]
        bqr_e = be1 @ g["w_qr"][:, qrs] + g["b_qr"][qrs]
        wkr_e = g1[:, None] * g["w_kr"]              # [C, 64]
        bkr_e = be1 @ g["w_kr"] + g["b_kr"]          # [64]
        wkr2 = np.zeros((2, C, P), f32)
        wkr2[0, :, :DHR] = wkr_e
        wkr2[0, :, DHR:] = wkr_e
        bkrD = np.concatenate([bkr_e, bkr_e])         # [P]

        cf32 = np.concatenate([
            bq_e.reshape(4, P).T, bk_e.reshape(4, P).T, bv_e.reshape(4, P).T,
            bqr_e.reshape(2, P).T, bkrD[:, None], np.zeros((P, 1), f32),
            bo_t, bff1_t, bff2_t], axis=1)
        assert cf32.shape == (P, 112)

        xs = x[b].T                                  # [C, T]
        m = {
            "xbf": np.ascontiguousarray(
                xs.reshape(CC, P, 4, NT).transpose(2, 1, 0, 3).astype(f16)),
            "xown16": np.ascontiguousarray(
                xs[:, 512 * r:512 * (r + 1)].reshape(CC, P, NT)
                .transpose(1, 0, 2).astype(f16)),
            "cf32": np.ascontiguousarray(cf32.astype(f32)),
            "cf16": cf16, "mask": mask_t,
            "wq": np.ascontiguousarray(
                wq_e.reshape(CC, P, 4 * P).transpose(1, 0, 2).astype(f16)),
            "wk": np.ascontiguousarray(
                wk_e.reshape(CC, P, 4 * P).transpose(1, 0, 2).astype(f16)),
            "wv": np.ascontiguousarray(
                wv_e.reshape(CC, P, 4 * P).transpose(1, 0, 2).astype(f16)),
            "wqr": np.ascontiguousarray(
                wqr_e.reshape(CC, P, 2 * P).transpose(1, 0, 2).astype(f16)),
            "wkr": np.ascontiguousarray(
                wkr2.reshape(2, CC, P, P).transpose(0, 2, 1, 3).astype(f16)),
            "trig": trig,
            "wo": wo_t,
            "wff1": wff1_t,
            "wff2": wff2_t,
        }
        in_maps.append(m)
    return in_maps


def assemble_output(results):
    out = np.zeros((B, T, C), np.float32)
    for c in range(NCORES):
        b, r = divmod(c, 4)
        o = results[c]["outT"].reshape(C, NT)
        out[b, NT * r:NT * (r + 1), :] = o.T
    return out


def kernel(**inputs):
    from concourse import bass_utils
    nc = _CACHE.get("nc")
    if nc is None:
        nc = build_program(repeat=1)
        _CACHE["nc"] = nc
    in_maps = prepare_in_maps(inputs)
    res = bass_utils.run_bass_kernel_spmd(nc, in_maps,
                                          core_ids=list(range(NCORES)))
    return assemble_output(res.results)
